# revision 1
# baseline (speedup 1.0000x reference)
"""GAT (2-layer, PyG-style) on 8 Trainium2 NeuronCores.

Strategy (edge parallelism per the sharding hint: "shard edges and their
gathered src features across devices"):
  - Nodes are split into 8 contiguous ranges (12500/core); each core owns all
    in-edges of its nodes (~412K edges, uniform since the graph is random).
  - Host gathers x[src] per edge into a padded-CSR slot layout (node-per-
    partition-lane x degree-slot), so the device only does dense streaming:
    no indirect DMA, no masks, no collectives.
  - Per-core nodes are degree-sorted so each 128-node tile has a near-uniform
    degree; slots are padded to the per-tile max degree (1.4% padding).
    Slot j=0 of every real node is its self-loop, which yields a_dst.
  - Pad slots use a host-solved feature vector v with v.w_asrc = -BIG and
    v.w_adst = 0, so exp(leakyrelu(logit)) == 0 exactly: pads vanish.
  - Layer 1 on device: he = xe @ [W1 | W1@Asrc | W1@Adst] (PE, bf16),
    e = exp(lrelu(a_src + a_dst)) (ACT), V = e*h (DVE), segment-sum = free-dim
    reduce over the degree axis (DVE), normalize, +b1, ELU, then
    R2 = [h2 | a_src2 | a_dst2] = elu_out @ [W2 | W2@Asrc2 | W2@Adst2].
  - Host round-trip: gather R2[src] per edge slot (12B/edge), second launch
    does layer 2 the same way + log_softmax.
"""

import sys

sys.path.insert(0, "/opt/trn_rl_repo")

import re
from contextlib import ExitStack

import ml_dtypes
import numpy as np

import concourse.tile as tile
from concourse import bass, mybir
from concourse.bass_utils import run_bass_kernel_spmd
from concourse.masks import make_identity

F32 = mybir.dt.float32
BF16 = mybir.dt.bfloat16
BF = ml_dtypes.bfloat16

NC = 8
TILE = 128
G1 = 14  # layer-1 j-group (14*36 fp32 = 2016B -> one PSUM bank)
G2 = 32  # layer-2 j-group
NEG_SLOPE = 0.2
BIG_NEG = -1.0e6


_ws_seq = [0]


def _split_waits(nc, limit=1):
    """The walrus build in this container rejects instructions carrying more
    than one sem wait ("Too many sync wait commands"). Hoist excess waits
    onto NOP carriers inserted just before the instruction (same engine, same
    program order, so semantics are preserved)."""
    for f in nc.m.functions:
        for blk in f.blocks:
            il = list(blk.instructions)
            out = []
            changed = False
            for inst in il:
                si = inst.sync_info
                waits = list(si.on_wait) if (si and si.on_wait) else []
                if len(waits) > limit:
                    keep = waits[-limit:]
                    for w in waits[:-limit]:
                        _ws_seq[0] += 1
                        nop = mybir.InstNoOp(name=f"WS-{_ws_seq[0]}")
                        nop.engine = inst.engine
                        nop.sync_info = mybir.SyncInfo(on_wait=[w], on_update=[])
                        out.append(nop)
                    si.on_wait = keep
                    changed = True
                out.append(inst)
            if changed:
                blk.instructions = out


# ---------------------------------------------------------------- host prep


def _plan(src, dst, n_nodes, n_cores):
    """Node ranges, degree-sorted tiles, shared D_t schedule, slot src ids."""
    per = n_nodes // n_cores
    ntiles = (per + TILE - 1) // TILE
    padn = ntiles * TILE

    deg = np.bincount(dst, minlength=n_nodes)

    # edges sorted by dst, self-loop (src==dst) first within each segment
    order_e = np.lexsort((src != dst, dst))
    s_src = src[order_e]
    rowptr = np.zeros(n_nodes + 1, dtype=np.int64)
    np.cumsum(deg, out=rowptr[1:])

    orders = []  # per core: global node id per sorted slot lane (-1 = fake)
    Dt_all = np.zeros((n_cores, ntiles), dtype=np.int64)
    for c in range(n_cores):
        d = deg[c * per : (c + 1) * per]
        ids = np.concatenate(
            [c * per + np.arange(per), np.full(padn - per, -1, np.int64)]
        )
        dd = np.concatenate([d, np.zeros(padn - per, np.int64)])
        o = np.argsort(dd, kind="stable")
        orders.append(ids[o])
        Dt_all[c] = dd[o].reshape(ntiles, TILE).max(axis=1)
    Dt = Dt_all.max(axis=0)
    Dt = np.maximum(Dt, 1)  # avoid zero-size tiles
    nblocks = int(Dt.sum())

    # slot src ids per core: [nblocks, TILE] int64, pad = n_nodes
    slot_src = np.full((n_cores, nblocks, TILE), n_nodes, dtype=np.int64)
    for c in range(n_cores):
        ids = orders[c]
        b0 = 0
        for t in range(ntiles):
            D = int(Dt[t])
            nid = ids[t * TILE : (t + 1) * TILE]
            real = nid >= 0
            nid_c = np.where(real, nid, 0)
            degs = np.where(real, deg[nid_c], 0)
            jj = np.arange(D)[:, None]  # [D, TILE]
            valid = jj < degs[None, :]
            eidx = rowptr[nid_c][None, :] + np.minimum(jj, np.maximum(degs - 1, 0))
            vals = s_src[np.clip(eidx, 0, len(s_src) - 1)]
            slot_src[c, b0 : b0 + D] = np.where(valid, vals, n_nodes)
            b0 += D
    return per, ntiles, padn, Dt, nblocks, slot_src, orders


def _pad_vector(W1, att_src1, att_dst1):
    """v with v.w_asrc_h = BIG_NEG and v.w_adst_h = 0 for both heads."""
    H, C = att_src1.shape
    cons = []
    rhs = []
    for h in range(H):
        cons.append(W1[:, h * C : (h + 1) * C] @ att_src1[h])
        rhs.append(BIG_NEG)
    for h in range(H):
        cons.append(W1[:, h * C : (h + 1) * C] @ att_dst1[h])
        rhs.append(0.0)
    A = np.stack(cons).astype(np.float64)  # [2H, F]
    v, *_ = np.linalg.lstsq(A, np.array(rhs, np.float64), rcond=None)
    return v.astype(np.float32)


# ------------------------------------------------------------- launch 1 (L1)


def _build_l1(nblocks, ntiles, Dt, padn, fdim, rec, nh, ch, repeat=None):
    """he = xe@W1p; e = exp(lrelu(a_src + a_dst)); V = [e*h | e];
    acc = sum_j V; out1 = acc[:, :2h*c]/s + b1; elu; R2 = eluT@W2p."""
    d1 = nh * ch  # 32
    nc = bass.Bass("TRN2")
    xet = nc.declare_dram_parameter("xet", [fdim, nblocks, TILE], BF16, isOutput=False)
    w1p = nc.declare_dram_parameter("w1p", [fdim, rec], BF16, isOutput=False)
    b1r = nc.declare_dram_parameter("b1r", [TILE, d1], F32, isOutput=False)
    w2p = nc.declare_dram_parameter("w2p", [d1, 4], F32, isOutput=False)
    r2 = nc.declare_dram_parameter("r2", [padn, 4], F32, isOutput=True)

    with ExitStack() as ctx:
        tc = ctx.enter_context(tile.TileContext(nc))
        const = ctx.enter_context(tc.tile_pool(name="const", bufs=1))
        xe = ctx.enter_context(tc.tile_pool(name="xe", bufs=6))
        hpool = ctx.enter_context(tc.tile_pool(name="hp", bufs=3, space="PSUM"))
        ppool = ctx.enter_context(tc.tile_pool(name="pp", bufs=2, space="PSUM"))
        vpool = ctx.enter_context(tc.tile_pool(name="vp", bufs=2))
        work = ctx.enter_context(tc.tile_pool(name="wk", bufs=2))
        outp = ctx.enter_context(tc.tile_pool(name="op", bufs=2))

        w1t = const.tile([fdim, rec], BF16)
        nc.sync.dma_start(out=w1t[:], in_=w1p[:])
        b1t = const.tile([TILE, d1], F32)
        nc.sync.dma_start(out=b1t[:], in_=b1r[:])
        w2t = const.tile([d1, 4], F32)
        nc.sync.dma_start(out=w2t[:], in_=w2p[:])
        ident = const.tile([TILE, TILE], F32)
        make_identity(nc, ident[:])

        if repeat:
            ctx.enter_context(tc.For_i(0, repeat, 1))
        accb = vpool.tile([TILE, ntiles, rec - 2], F32, tag="accb")
        blk = 0
        for t in range(ntiles):
            D = int(Dt[t])
            V = vpool.tile([TILE, D, rec - 2], F32, tag="V")  # [h*e | e] = 34
            et = work.tile([TILE, nh, D], F32, tag="et")
            adst = work.tile([TILE, nh], F32, tag="adst")
            for g0 in range(0, D, G1):
                g = min(G1, D - g0)
                xt = xe.tile([TILE, G1, TILE], BF16, tag="xt")
                nc.sync.dma_start(
                    out=xt[:, 0:g, :], in_=xet[:, blk : blk + g, :]
                )
                hp = hpool.tile([TILE, G1, rec], F32, tag="hp")
                for j in range(g):
                    nc.tensor.matmul(
                        out=hp[:, j, :],
                        lhsT=xt[:, j, :],
                        rhs=w1t[:],
                        start=True,
                        stop=True,
                    )
                if g0 == 0:
                    nc.vector.tensor_copy(out=adst[:], in_=hp[:, 0, d1 + nh : rec])
                lg = work.tile([TILE, G1, nh], F32, tag="lg")
                nc.vector.tensor_tensor(
                    out=lg[:, 0:g, :],
                    in0=hp[:, 0:g, d1 : d1 + nh],
                    in1=adst[:].unsqueeze(1).to_broadcast([TILE, g, nh]),
                    op=mybir.AluOpType.add,
                )
                ls = work.tile([TILE, G1, nh], F32, tag="ls")
                nc.vector.tensor_scalar_mul(
                    out=ls[:, 0:g, :], in0=lg[:, 0:g, :], scalar1=NEG_SLOPE
                )
                nc.vector.tensor_tensor(
                    out=lg[:, 0:g, :],
                    in0=lg[:, 0:g, :],
                    in1=ls[:, 0:g, :],
                    op=mybir.AluOpType.max,
                )
                nc.scalar.activation(
                    out=et[:, :, g0 : g0 + g],
                    in_=lg[:, 0:g, :].rearrange("p g h -> p h g"),
                    func=mybir.ActivationFunctionType.Exp,
                )
                for h in range(nh):
                    nc.vector.tensor_tensor(
                        out=V[:, g0 : g0 + g, h * ch : (h + 1) * ch],
                        in0=hp[:, 0:g, h * ch : (h + 1) * ch],
                        in1=et[:, h, g0 : g0 + g].unsqueeze(-1).to_broadcast(
                            [TILE, g, ch]
                        ),
                        op=mybir.AluOpType.mult,
                    )
                    nc.vector.tensor_copy(
                        out=V[:, g0 : g0 + g, d1 + h],
                        in_=et[:, h, g0 : g0 + g],
                    )
                blk += g

            nc.vector.tensor_reduce(
                out=accb[:, t, :],
                in_=V[:].rearrange("p j c -> p c j"),
                axis=mybir.AxisListType.X,
                op=mybir.AluOpType.add,
            )

        # ---- batched finishing across all tiles ----
        inv = work.tile([TILE, ntiles, nh], F32, tag="inv")
        nc.vector.tensor_scalar_add(
            out=inv[:], in0=accb[:, :, d1 : d1 + nh], scalar1=1e-16
        )
        nc.vector.reciprocal(out=inv[:], in_=inv[:])
        o1a = vpool.tile([TILE, ntiles, d1], F32, tag="o1a")
        nc.vector.tensor_tensor(
            out=o1a[:].rearrange("p t (h c) -> p t h c", h=nh),
            in0=accb[:, :, 0:d1].rearrange("p t (h c) -> p t h c", h=nh),
            in1=inv[:].unsqueeze(-1).to_broadcast([TILE, ntiles, nh, ch]),
            op=mybir.AluOpType.mult,
        )
        nc.vector.tensor_tensor(
            out=o1a[:],
            in0=o1a[:],
            in1=b1t[:].unsqueeze(1).to_broadcast([TILE, ntiles, d1]),
            op=mybir.AluOpType.add,
        )
        # elu = max(x,0) + exp(min(x,0)) - 1
        e1 = vpool.tile([TILE, ntiles, d1], F32, tag="e1")
        nc.vector.tensor_scalar_min(out=e1[:], in0=o1a[:], scalar1=0.0)
        nc.scalar.activation(
            out=e1[:], in_=e1[:], func=mybir.ActivationFunctionType.Exp
        )
        nc.vector.tensor_scalar_add(out=e1[:], in0=e1[:], scalar1=-1.0)
        nc.vector.tensor_scalar_max(out=o1a[:], in0=o1a[:], scalar1=0.0)
        nc.vector.tensor_tensor(
            out=o1a[:], in0=o1a[:], in1=e1[:], op=mybir.AluOpType.add
        )
        # R2 = [h2 | a_src2 | a_dst2] = (elu_out)^T.T @ w2p, staged per tile
        r2all = outp.tile([TILE, ntiles, 4], F32, tag="r2all")
        for t in range(ntiles):
            pt = ppool.tile([d1, TILE], F32, tag="pt")
            nc.tensor.transpose(out=pt[:], in_=o1a[:, t, :], identity=ident[:])
            o1t = work.tile([d1, TILE], F32, tag="o1t")
            nc.vector.tensor_copy(out=o1t[:], in_=pt[:])
            r2p = ppool.tile([TILE, 4], F32, tag="r2p")
            nc.tensor.matmul(
                out=r2p[:], lhsT=o1t[:], rhs=w2t[:], start=True, stop=True
            )
            nc.vector.tensor_copy(out=r2all[:, t, :], in_=r2p[:])
        nc.sync.dma_start(
            out=r2[:].rearrange("(t n) c -> n t c", n=TILE), in_=r2all[:]
        )
    return nc


# ------------------------------------------------------------- launch 2 (L2)


def _build_l2(nblocks, ntiles, Dt, padn, repeat=None):
    """Layer 2 (1 head, 2 ch) from host-gathered [h2(2) | a_src2] slots,
    plus bias and log_softmax."""
    nc = bass.Bass("TRN2")
    xe2 = nc.declare_dram_parameter("xe2", [TILE, nblocks, 4], BF16, isOutput=False)
    ad2 = nc.declare_dram_parameter("ad2", [padn, 1], F32, isOutput=False)
    b2r = nc.declare_dram_parameter("b2r", [TILE, 2], F32, isOutput=False)
    y = nc.declare_dram_parameter("y", [padn, 2], F32, isOutput=True)

    with ExitStack() as ctx:
        tc = ctx.enter_context(tile.TileContext(nc))
        const = ctx.enter_context(tc.tile_pool(name="const", bufs=1))
        xe = ctx.enter_context(tc.tile_pool(name="xe", bufs=6))
        vpool = ctx.enter_context(tc.tile_pool(name="vp", bufs=2))
        work = ctx.enter_context(tc.tile_pool(name="wk", bufs=2))
        outp = ctx.enter_context(tc.tile_pool(name="op", bufs=2))

        b2t = const.tile([TILE, 2], F32)
        nc.sync.dma_start(out=b2t[:], in_=b2r[:])

        adall = const.tile([TILE, ntiles], F32)
        nc.sync.dma_start(
            out=adall[:], in_=ad2[:].rearrange("(t n) one -> n (t one)", n=TILE)
        )
        if repeat:
            ctx.enter_context(tc.For_i(0, repeat, 1))
        accb = vpool.tile([TILE, ntiles, 3], F32, tag="accb")
        blk = 0
        for t in range(ntiles):
            D = int(Dt[t])
            V = vpool.tile([TILE, D, 3], F32, tag="V")
            for g0 in range(0, D, G2):
                g = min(G2, D - g0)
                xt = xe.tile([TILE, G2, 4], BF16, tag="xt")
                nc.sync.dma_start(out=xt[:, 0:g, :], in_=xe2[:, blk : blk + g, :])
                h2f = work.tile([TILE, G2, 2], F32, tag="h2f")
                nc.vector.tensor_copy(out=h2f[:, 0:g, :], in_=xt[:, 0:g, 0:2])
                lr = work.tile([TILE, G2], F32, tag="lr")
                nc.vector.tensor_copy(out=lr[:, 0:g], in_=xt[:, 0:g, 2])
                nc.vector.tensor_tensor(
                    out=lr[:, 0:g],
                    in0=lr[:, 0:g],
                    in1=adall[:, t : t + 1].to_broadcast([TILE, g]),
                    op=mybir.AluOpType.add,
                )
                ls2 = work.tile([TILE, G2], F32, tag="ls2")
                nc.vector.tensor_scalar_mul(
                    out=ls2[:, 0:g], in0=lr[:, 0:g], scalar1=NEG_SLOPE
                )
                nc.vector.tensor_tensor(
                    out=lr[:, 0:g],
                    in0=lr[:, 0:g],
                    in1=ls2[:, 0:g],
                    op=mybir.AluOpType.max,
                )
                e2 = work.tile([TILE, G2], F32, tag="e2")
                nc.scalar.activation(
                    out=e2[:, 0:g],
                    in_=lr[:, 0:g],
                    func=mybir.ActivationFunctionType.Exp,
                )
                nc.vector.tensor_tensor(
                    out=V[:, g0 : g0 + g, 0:2],
                    in0=h2f[:, 0:g, :],
                    in1=e2[:, 0:g].unsqueeze(-1).to_broadcast([TILE, g, 2]),
                    op=mybir.AluOpType.mult,
                )
                nc.vector.tensor_copy(out=V[:, g0 : g0 + g, 2], in_=e2[:, 0:g])
                blk += g

            nc.vector.tensor_reduce(
                out=accb[:, t, :],
                in_=V[:].rearrange("p j c -> p c j"),
                axis=mybir.AxisListType.X,
                op=mybir.AluOpType.add,
            )

        # ---- batched finishing across all tiles ----
        inv = work.tile([TILE, ntiles], F32, tag="inv")
        nc.vector.tensor_scalar_add(out=inv[:], in0=accb[:, :, 2], scalar1=1e-16)
        nc.vector.reciprocal(out=inv[:], in_=inv[:])
        z = vpool.tile([TILE, ntiles, 2], F32, tag="z")
        nc.vector.tensor_tensor(
            out=z[:],
            in0=accb[:, :, 0:2],
            in1=inv[:].unsqueeze(-1).to_broadcast([TILE, ntiles, 2]),
            op=mybir.AluOpType.mult,
        )
        nc.vector.tensor_tensor(
            out=z[:],
            in0=z[:],
            in1=b2t[:].unsqueeze(1).to_broadcast([TILE, ntiles, 2]),
            op=mybir.AluOpType.add,
        )
        # log_softmax over the 2 columns
        m = work.tile([TILE, ntiles], F32, tag="m")
        nc.vector.tensor_reduce(
            out=m[:], in_=z[:], axis=mybir.AxisListType.X, op=mybir.AluOpType.max
        )
        nc.vector.tensor_tensor(
            out=z[:],
            in0=z[:],
            in1=m[:].unsqueeze(-1).to_broadcast([TILE, ntiles, 2]),
            op=mybir.AluOpType.subtract,
        )
        ez = vpool.tile([TILE, ntiles, 2], F32, tag="ez")
        nc.scalar.activation(
            out=ez[:], in_=z[:], func=mybir.ActivationFunctionType.Exp
        )
        ss = work.tile([TILE, ntiles], F32, tag="ss")
        nc.vector.tensor_reduce(
            out=ss[:],
            in_=ez[:],
            axis=mybir.AxisListType.X,
            op=mybir.AluOpType.add,
        )
        nc.scalar.activation(
            out=ss[:], in_=ss[:], func=mybir.ActivationFunctionType.Ln
        )
        yt = outp.tile([TILE, ntiles, 2], F32, tag="yt")
        nc.vector.tensor_tensor(
            out=yt[:],
            in0=z[:],
            in1=ss[:].unsqueeze(-1).to_broadcast([TILE, ntiles, 2]),
            op=mybir.AluOpType.subtract,
        )
        nc.sync.dma_start(
            out=y[:].rearrange("(t n) c -> n t c", n=TILE), in_=yt[:]
        )
    return nc


# ------------------------------------------------------------------- driver


def _run_gat(x, edge_index, W1, att_src1, att_dst1, b1, W2, att_src2, att_dst2, b2,
             n_cores=NC, timing=None):
    n_nodes, fdim = x.shape
    nh, ch = att_src1.shape
    d1 = nh * ch
    rec = d1 + 2 * nh  # h | a_src | a_dst

    src = np.concatenate([np.asarray(edge_index[0]), np.arange(n_nodes)]).astype(
        np.int64
    )
    dst = np.concatenate([np.asarray(edge_index[1]), np.arange(n_nodes)]).astype(
        np.int64
    )

    per, ntiles, padn, Dt, nblocks, slot_src, orders = _plan(
        src, dst, n_nodes, n_cores
    )

    W1 = np.asarray(W1, np.float32)
    att_src1 = np.asarray(att_src1, np.float32)
    att_dst1 = np.asarray(att_dst1, np.float32)
    W2 = np.asarray(W2, np.float32)
    att_src2 = np.asarray(att_src2, np.float32)
    att_dst2 = np.asarray(att_dst2, np.float32)

    # fused weights
    w_asrc1 = np.stack(
        [W1[:, h * ch : (h + 1) * ch] @ att_src1[h] for h in range(nh)], axis=1
    )  # [F, nh]
    w_adst1 = np.stack(
        [W1[:, h * ch : (h + 1) * ch] @ att_dst1[h] for h in range(nh)], axis=1
    )
    w1p = np.concatenate([W1, w_asrc1, w_adst1], axis=1)  # [F, rec]
    nh2, ch2 = att_src2.shape  # 1, 2
    w_asrc2 = W2 @ att_src2[0]
    w_adst2 = W2 @ att_dst2[0]
    w2p = np.concatenate(
        [W2, w_asrc2[:, None], w_adst2[:, None]], axis=1
    ).astype(np.float32)  # [d1, 4]

    pad_vec = _pad_vector(W1, att_src1, att_dst1)
    x_ext = np.concatenate([np.asarray(x, np.float32), pad_vec[None]], axis=0).astype(
        BF
    )  # [n+1, F]

    # per-core L1 inputs: xet [F, nblocks, TILE] bf16, feature-major
    in_maps1 = []
    w1p_bf = w1p.astype(BF)
    b1r = np.broadcast_to(np.asarray(b1, np.float32), (TILE, d1)).copy()
    for c in range(n_cores):
        g = x_ext[slot_src[c].reshape(-1)]  # [nblocks*TILE, F]
        g = g.reshape(nblocks, TILE, fdim).transpose(2, 0, 1)  # [F, nb, TILE]
        in_maps1.append(
            {
                "xet": np.ascontiguousarray(g),
                "w1p": w1p_bf,
                "b1r": b1r,
                "w2p": w2p,
            }
        )

    nc1 = _build_l1(nblocks, ntiles, Dt, padn, fdim, rec, nh, ch)
    _split_waits(nc1)
    import time as _time

    t0 = _time.perf_counter()
    res1 = run_bass_kernel_spmd(nc1, in_maps1, list(range(n_cores)))
    t1 = _time.perf_counter()
    if timing is not None:
        timing["l1_first_s"] = t1 - t0
        timing["nc1"] = nc1
        timing["in_maps1"] = in_maps1

    # assemble R2 table and gather layer-2 slots on host
    h2tab = np.zeros((n_nodes + 1, 4), np.float32)
    h2tab[n_nodes] = [0.0, 0.0, BIG_NEG, 0.0]
    for c in range(n_cores):
        r2c = res1.results[c]["r2"]  # [padn, 4]
        ids = orders[c]
        real = ids >= 0
        h2tab[ids[real]] = r2c[real]

    in_maps2 = []
    for c in range(n_cores):
        vals = h2tab[slot_src[c].reshape(-1)][:, 0:3]  # [nb*TILE, 3]
        vals4 = np.zeros((nblocks * TILE, 4), np.float32)
        vals4[:, 0:3] = vals
        xe2 = (
            vals4.reshape(nblocks, TILE, 4).transpose(1, 0, 2).astype(BF)
        )  # [TILE, nb, 4] lane-major
        ids = orders[c]
        ad2 = np.where(ids >= 0, h2tab[np.maximum(ids, 0), 3], 0.0).astype(
            np.float32
        )[:, None]
        b2r = np.broadcast_to(np.asarray(b2, np.float32), (TILE, 2)).copy()
        in_maps2.append(
            {"xe2": np.ascontiguousarray(xe2), "ad2": ad2, "b2r": b2r}
        )

    nc2 = _build_l2(nblocks, ntiles, Dt, padn)
    _split_waits(nc2)
    t2 = _time.perf_counter()
    res2 = run_bass_kernel_spmd(nc2, in_maps2, list(range(n_cores)))
    t3 = _time.perf_counter()
    if timing is not None:
        timing["l2_first_s"] = t3 - t2
        timing["nc2"] = nc2
        timing["in_maps2"] = in_maps2

    out = np.zeros((n_nodes, 2), np.float32)
    for c in range(n_cores):
        yc = res2.results[c]["y"]
        ids = orders[c]
        real = ids >= 0
        out[ids[real]] = yc[real]
    return out


def kernel(x, edge_index, W1, att_src1, att_dst1, b1, W2, att_src2, att_dst2, b2):
    return _run_gat(
        np.asarray(x, np.float32),
        np.asarray(edge_index),
        W1,
        att_src1,
        att_dst1,
        b1,
        W2,
        att_src2,
        att_dst2,
        b2,
    )



# revision 4
# speedup vs baseline: 2.2739x; 2.2739x over previous
"""GAT (2-layer, PyG-style) on 8 Trainium2 NeuronCores.

Strategy (edge parallelism per the sharding hint), v2 — three launches:
  - Nodes are split into 8 contiguous ranges (12500/core); each core owns all
    in-edges of its nodes (~412K edges, uniform since the graph is random).
    Per-core nodes are degree-sorted into 128-lane tiles with a shared
    per-tile max-degree slot schedule Dt (1.2% padding).
  - Launch A (node pass): R1 = [h1 | a_src1 | a_dst1] = x @ [W1|W1@As|W1@Ad]
    computed once per node on the PE (fp16, weights stationary, output
    channel-major [36, nodes]). 3.2MB/core in, 0.9MB out.
  - Host gathers R1[src] per edge slot (72B/edge fp16 vs 256B bf16 for raw x
    in v1 -- 3.5x less HBM traffic, and no per-edge matmuls).
  - Launch B (layer-1 edge pass): per group of tiles, stream channel-major
    slot planes [h(32) | a_src(2) | a_dst(2)] fp16; DVE computes
    e = exp(lrelu(a_src + a_dst)) (exp on ACT), V = e*h (fp16, packed for the
    DVE 2x/4x fast paths), segment-sum = per-tile reduce over the slot axis
    (f32 accumulate); normalize, +b1, ELU, R2 = eluT @ [W2|W2@As2|W2@Ad2].
  - Host gathers R2[src] per edge slot (8B/edge), launch C does layer 2 the
    same way (single planar load, whole-core-sized instructions) +
    log_softmax.
"""

import sys

sys.path.insert(0, "/opt/trn_rl_repo")

from contextlib import ExitStack

import numpy as np

import concourse.tile as tile
from concourse import bass, mybir
from concourse.bass_utils import run_bass_kernel_spmd
from concourse.masks import make_identity

F32 = mybir.dt.float32
F16 = mybir.dt.float16
NP16 = np.float16

NC = 8
TILE = 128
NH = 2
CH = 16
D1 = NH * CH  # 32
REC = D1 + 2 * NH  # 36
NEG_SLOPE = 0.2
BIG_NEG = -30000.0  # fp16-safe; 0.2*BIG_NEG underflows exp to exactly 0
CG_BUDGET = 384  # max slot columns per launch-B group (SBUF bound)


_ws_seq = [0]


def _split_waits(nc, limit=1):
    """The walrus build in this container rejects instructions carrying more
    than one sem wait ("Too many sync wait commands"). Hoist excess waits
    onto NOP carriers inserted just before the instruction (same engine, same
    program order, so semantics are preserved)."""
    for f in nc.m.functions:
        for blk in f.blocks:
            il = list(blk.instructions)
            out = []
            changed = False
            for inst in il:
                si = inst.sync_info
                waits = list(si.on_wait) if (si and si.on_wait) else []
                if len(waits) > limit:
                    keep = waits[-limit:]
                    for w in waits[:-limit]:
                        _ws_seq[0] += 1
                        nop = mybir.InstNoOp(name=f"WS-{_ws_seq[0]}")
                        nop.engine = inst.engine
                        nop.sync_info = mybir.SyncInfo(on_wait=[w], on_update=[])
                        out.append(nop)
                    si.on_wait = keep
                    changed = True
                out.append(inst)
            if changed:
                blk.instructions = out


# ---------------------------------------------------------------- host prep


def _plan(src, dst, n_nodes, n_cores):
    """Node ranges, degree-sorted tiles, shared D_t schedule, slot src ids."""
    per = n_nodes // n_cores
    ntiles = (per + TILE - 1) // TILE
    padn = ntiles * TILE

    deg = np.bincount(dst, minlength=n_nodes)

    # edges sorted by dst, self-loop (src==dst) first within each segment
    order_e = np.lexsort((src != dst, dst))
    s_src = src[order_e]
    rowptr = np.zeros(n_nodes + 1, dtype=np.int64)
    np.cumsum(deg, out=rowptr[1:])

    orders = []  # per core: global node id per sorted slot lane (-1 = fake)
    Dt_all = np.zeros((n_cores, ntiles), dtype=np.int64)
    for c in range(n_cores):
        d = deg[c * per : (c + 1) * per]
        ids = np.concatenate(
            [c * per + np.arange(per), np.full(padn - per, -1, np.int64)]
        )
        dd = np.concatenate([d, np.zeros(padn - per, np.int64)])
        o = np.argsort(dd, kind="stable")
        orders.append(ids[o])
        Dt_all[c] = dd[o].reshape(ntiles, TILE).max(axis=1)
    Dt = Dt_all.max(axis=0)
    Dt = np.maximum(Dt, 1)  # avoid zero-size tiles
    nblocks = int(Dt.sum())

    # slot src ids per core: [nblocks, TILE] int64, pad = n_nodes
    slot_src = np.full((n_cores, nblocks, TILE), n_nodes, dtype=np.int64)
    for c in range(n_cores):
        ids = orders[c]
        b0 = 0
        for t in range(ntiles):
            D = int(Dt[t])
            nid = ids[t * TILE : (t + 1) * TILE]
            real = nid >= 0
            nid_c = np.where(real, nid, 0)
            degs = np.where(real, deg[nid_c], 0)
            jj = np.arange(D)[:, None]  # [D, TILE]
            valid = jj < degs[None, :]
            eidx = rowptr[nid_c][None, :] + np.minimum(jj, np.maximum(degs - 1, 0))
            vals = s_src[np.clip(eidx, 0, len(s_src) - 1)]
            slot_src[c, b0 : b0 + D] = np.where(valid, vals, n_nodes)
            b0 += D
    return per, ntiles, padn, Dt, nblocks, slot_src, orders


def _groups(Dt):
    """Pack degree-sorted tiles into groups with bounded total slot columns."""
    groups = []  # (t0, n_tiles, CG)
    t0 = 0
    cg = 0
    for t, d in enumerate(Dt):
        d = int(d)
        if cg + d > CG_BUDGET and t > t0:
            groups.append((t0, t - t0, cg))
            t0, cg = t, 0
        cg += d
    groups.append((t0, len(Dt) - t0, cg))
    return groups


# ------------------------------------------------------- launch A (node pass)


def _build_a(padn, fdim, repeat=None):
    """R1 = [h1 | a_src1 | a_dst1] = w1p.T @ x, channel-major out [REC, padn]."""
    nc = bass.Bass("TRN2")
    xt = nc.declare_dram_parameter("xt", [fdim, padn], F16, isOutput=False)
    w1p = nc.declare_dram_parameter("w1p", [fdim, REC], F16, isOutput=False)
    r1 = nc.declare_dram_parameter("r1", [REC, padn], F16, isOutput=True)
    nt = padn // TILE

    with ExitStack() as ctx:
        tc = ctx.enter_context(tile.TileContext(nc))
        const = ctx.enter_context(tc.tile_pool(name="const", bufs=1))
        xp = ctx.enter_context(tc.tile_pool(name="xp", bufs=1))
        pp = ctx.enter_context(tc.tile_pool(name="pp", bufs=4, space="PSUM"))
        op = ctx.enter_context(tc.tile_pool(name="op", bufs=1))

        w1t = const.tile([fdim, REC], F16)
        nc.sync.dma_start(out=w1t[:], in_=w1p[:])

        if repeat:
            ctx.enter_context(tc.For_i(0, repeat, 1))
        xtile = xp.tile([fdim, padn], F16, tag="xt")
        nc.sync.dma_start(out=xtile[:], in_=xt[:])
        r1sb = op.tile([REC, padn], F16, tag="r1sb")
        p1 = None
        for t in range(nt):
            q = t % 4
            if q == 0:
                p1 = pp.tile([REC, 4 * TILE], F32, tag="p1")
            nc.tensor.matmul(
                out=p1[:, q * TILE : (q + 1) * TILE],
                lhsT=w1t[:],
                rhs=xtile[:, t * TILE : (t + 1) * TILE],
                start=True,
                stop=True,
            )
            if q == 3 or t == nt - 1:
                nc.vector.tensor_copy(
                    out=r1sb[:, (t - q) * TILE : (t + 1) * TILE],
                    in_=p1[:, 0 : (q + 1) * TILE],
                )
        nc.sync.dma_start(out=r1[:], in_=r1sb[:])
    return nc


# ------------------------------------------------------------- launch B (L1)


def _build_l1(Dt, groups, ntiles, padn, repeat=None):
    """Layer-1 edge pass from host-gathered channel-major slot planes."""
    cb = 36 * int(Dt.sum())
    cgm = max(g[2] for g in groups)
    nc = bass.Bass("TRN2")
    hsd = nc.declare_dram_parameter("hsd", [TILE, cb], F16, isOutput=False)
    b1r = nc.declare_dram_parameter("b1r", [TILE, D1], F16, isOutput=False)
    w2p = nc.declare_dram_parameter("w2p", [D1, 4], F16, isOutput=False)
    r2 = nc.declare_dram_parameter("r2", [padn, 4], F16, isOutput=True)

    with ExitStack() as ctx:
        tc = ctx.enter_context(tile.TileContext(nc))
        const = ctx.enter_context(tc.tile_pool(name="const", bufs=1))
        hspool = ctx.enter_context(tc.tile_pool(name="hs", bufs=2))
        wk = ctx.enter_context(tc.tile_pool(name="wk", bufs=2))
        vpool = ctx.enter_context(tc.tile_pool(name="vp", bufs=1))
        ppool = ctx.enter_context(tc.tile_pool(name="pp", bufs=2, space="PSUM"))
        rpool = ctx.enter_context(tc.tile_pool(name="rp", bufs=2, space="PSUM"))
        outp = ctx.enter_context(tc.tile_pool(name="op", bufs=1))

        b1t = const.tile([TILE, D1], F16)
        nc.sync.dma_start(out=b1t[:], in_=b1r[:])
        w2t = const.tile([D1, 4], F16)
        nc.sync.dma_start(out=w2t[:], in_=w2p[:])
        ident = const.tile([TILE, TILE], F16)
        make_identity(nc, ident[:])

        if repeat:
            ctx.enter_context(tc.For_i(0, repeat, 1))
        accb = vpool.tile([TILE, ntiles, D1], F32, tag="accb")
        sb = vpool.tile([TILE, ntiles, NH], F32, tag="sb")
        off = 0
        for t0, ng, cg in groups:
            hst = hspool.tile([TILE, 36, cgm], F16, tag="hst")
            nc.sync.dma_start(
                out=hst[:, :, 0:cg],
                in_=hsd[:, off : off + 36 * cg].rearrange("p (c j) -> p c j", c=36),
            )
            lg = wk.tile([TILE, NH, cgm], F16, tag="lg")
            nc.vector.tensor_tensor(
                out=lg[:, :, 0:cg],
                in0=hst[:, D1 : D1 + NH, 0:cg],
                in1=hst[:, D1 + NH : REC, 0:cg],
                op=mybir.AluOpType.add,
            )
            ls = wk.tile([TILE, NH, cgm], F16, tag="ls")
            nc.vector.tensor_scalar_mul(
                out=ls[:, :, 0:cg], in0=lg[:, :, 0:cg], scalar1=NEG_SLOPE
            )
            nc.vector.tensor_tensor(
                out=lg[:, :, 0:cg],
                in0=lg[:, :, 0:cg],
                in1=ls[:, :, 0:cg],
                op=mybir.AluOpType.max,
            )
            et = wk.tile([TILE, NH, cgm], F16, tag="et")
            nc.scalar.activation(
                out=et[:, :, 0:cg],
                in_=lg[:, :, 0:cg],
                func=mybir.ActivationFunctionType.Exp,
            )
            V = vpool.tile([TILE, NH, CH, cgm], F16, tag="V")
            nc.vector.tensor_tensor(
                out=V[:, :, :, 0:cg],
                in0=hst[:, 0:D1, 0:cg].rearrange("p (h c) j -> p h c j", h=NH),
                in1=et[:, :, 0:cg].unsqueeze(2).to_broadcast([TILE, NH, CH, cg]),
                op=mybir.AluOpType.mult,
            )
            d0 = 0
            for t in range(t0, t0 + ng):
                D = int(Dt[t])
                nc.vector.tensor_reduce(
                    out=accb[:, t, :].rearrange("p (h c) -> p h c", h=NH),
                    in_=V[:, :, :, d0 : d0 + D],
                    axis=mybir.AxisListType.X,
                    op=mybir.AluOpType.add,
                )
                nc.vector.tensor_reduce(
                    out=sb[:, t, :],
                    in_=et[:, :, d0 : d0 + D],
                    axis=mybir.AxisListType.X,
                    op=mybir.AluOpType.add,
                )
                d0 += D
            off += 36 * cg

        # ---- batched finishing across all tiles ----
        inv = outp.tile([TILE, ntiles, NH], F32, tag="inv")
        nc.vector.tensor_scalar_add(out=inv[:], in0=sb[:], scalar1=1e-16)
        nc.vector.reciprocal(out=inv[:], in_=inv[:])
        o1a = outp.tile([TILE, ntiles, D1], F16, tag="o1a")
        nc.vector.tensor_tensor(
            out=o1a[:].rearrange("p t (h c) -> p t h c", h=NH),
            in0=accb[:].rearrange("p t (h c) -> p t h c", h=NH),
            in1=inv[:].unsqueeze(-1).to_broadcast([TILE, ntiles, NH, CH]),
            op=mybir.AluOpType.mult,
        )
        nc.vector.tensor_tensor(
            out=o1a[:],
            in0=o1a[:],
            in1=b1t[:].unsqueeze(1).to_broadcast([TILE, ntiles, D1]),
            op=mybir.AluOpType.add,
        )
        # elu = max(x,0) + exp(min(x,0)) - 1
        e1 = outp.tile([TILE, ntiles, D1], F16, tag="e1")
        nc.vector.tensor_scalar_min(out=e1[:], in0=o1a[:], scalar1=0.0)
        nc.scalar.activation(
            out=e1[:], in_=e1[:], func=mybir.ActivationFunctionType.Exp
        )
        nc.vector.tensor_scalar_add(out=e1[:], in0=e1[:], scalar1=-1.0)
        nc.vector.tensor_scalar_max(out=o1a[:], in0=o1a[:], scalar1=0.0)
        nc.vector.tensor_tensor(
            out=o1a[:], in0=o1a[:], in1=e1[:], op=mybir.AluOpType.add
        )
        # R2 = [h2 | a_src2 | a_dst2] = elu_out @ w2p via PE transposes
        o1tsb = outp.tile([D1, padn], F16, tag="o1t")
        pt = None
        for t in range(ntiles):
            q = t % 4
            if q == 0:
                pt = ppool.tile([D1, 4 * TILE], F16, tag="pt")
            nc.tensor.transpose(
                out=pt[:, q * TILE : (q + 1) * TILE],
                in_=o1a[:, t, :],
                identity=ident[:],
            )
            if q == 3 or t == ntiles - 1:
                nc.vector.tensor_copy(
                    out=o1tsb[:, (t - q) * TILE : (t + 1) * TILE],
                    in_=pt[:, 0 : (q + 1) * TILE],
                )
        r2all = outp.tile([TILE, ntiles, 4], F16, tag="r2all")
        r2p = None
        for t in range(ntiles):
            q = t % 32
            if q == 0:
                r2p = rpool.tile([TILE, 32 * 4], F32, tag="r2p")
            nc.tensor.matmul(
                out=r2p[:, q * 4 : (q + 1) * 4],
                lhsT=o1tsb[:, t * TILE : (t + 1) * TILE],
                rhs=w2t[:],
                start=True,
                stop=True,
            )
            if q == 31 or t == ntiles - 1:
                nc.vector.tensor_copy(
                    out=r2all[:, t - q : t + 1, :],
                    in_=r2p[:, 0 : (q + 1) * 4].rearrange("p (t c) -> p t c", c=4),
                )
        nc.sync.dma_start(
            out=r2[:].rearrange("(t n) c -> n t c", n=TILE), in_=r2all[:]
        )
    return nc


# ------------------------------------------------------------- launch C (L2)


def _build_l2(Dt, ntiles, padn, repeat=None):
    """Layer 2 (1 head, 2 ch) from planar [h2(2) | a_src2 | a_dst2] slots,
    plus bias and log_softmax."""
    nb = int(Dt.sum())
    nc = bass.Bass("TRN2")
    xed = nc.declare_dram_parameter("xed", [TILE, 4 * nb], F16, isOutput=False)
    b2r = nc.declare_dram_parameter("b2r", [TILE, 2], F32, isOutput=False)
    y = nc.declare_dram_parameter("y", [padn, 2], F32, isOutput=True)

    with ExitStack() as ctx:
        tc = ctx.enter_context(tile.TileContext(nc))
        const = ctx.enter_context(tc.tile_pool(name="const", bufs=1))
        xp = ctx.enter_context(tc.tile_pool(name="xp", bufs=1))
        wk = ctx.enter_context(tc.tile_pool(name="wk", bufs=1))
        outp = ctx.enter_context(tc.tile_pool(name="op", bufs=1))

        b2t = const.tile([TILE, 2], F32)
        nc.sync.dma_start(out=b2t[:], in_=b2r[:])

        if repeat:
            ctx.enter_context(tc.For_i(0, repeat, 1))
        xe = xp.tile([TILE, 4 * nb], F16, tag="xe")
        nc.sync.dma_start(out=xe[:], in_=xed[:])
        h2 = xe[:, 0 : 2 * nb].rearrange("p (c j) -> p c j", c=2)
        as2 = xe[:, 2 * nb : 3 * nb]
        ad2 = xe[:, 3 * nb : 4 * nb]

        lg = wk.tile([TILE, nb], F16, tag="lg")
        nc.vector.tensor_tensor(
            out=lg[:], in0=as2, in1=ad2, op=mybir.AluOpType.add
        )
        ls = wk.tile([TILE, nb], F16, tag="ls")
        nc.vector.tensor_scalar_mul(out=ls[:], in0=lg[:], scalar1=NEG_SLOPE)
        nc.vector.tensor_tensor(
            out=lg[:], in0=lg[:], in1=ls[:], op=mybir.AluOpType.max
        )
        et = wk.tile([TILE, nb], F16, tag="et")
        nc.scalar.activation(
            out=et[:], in_=lg[:], func=mybir.ActivationFunctionType.Exp
        )
        V = wk.tile([TILE, 2, nb], F16, tag="V")
        nc.vector.tensor_tensor(
            out=V[:],
            in0=h2,
            in1=et[:].unsqueeze(1).to_broadcast([TILE, 2, nb]),
            op=mybir.AluOpType.mult,
        )
        acc2 = wk.tile([TILE, ntiles, 2], F32, tag="acc2")
        s2 = wk.tile([TILE, ntiles], F32, tag="s2")
        blk = 0
        for t in range(ntiles):
            D = int(Dt[t])
            nc.vector.tensor_reduce(
                out=acc2[:, t, :],
                in_=V[:, :, blk : blk + D],
                axis=mybir.AxisListType.X,
                op=mybir.AluOpType.add,
            )
            nc.vector.tensor_reduce(
                out=s2[:, t : t + 1],
                in_=et[:, blk : blk + D],
                axis=mybir.AxisListType.X,
                op=mybir.AluOpType.add,
            )
            blk += D

        # ---- batched finishing ----
        nc.vector.tensor_scalar_add(out=s2[:], in0=s2[:], scalar1=1e-16)
        nc.vector.reciprocal(out=s2[:], in_=s2[:])
        z = outp.tile([TILE, ntiles, 2], F32, tag="z")
        nc.vector.tensor_tensor(
            out=z[:],
            in0=acc2[:],
            in1=s2[:].unsqueeze(-1).to_broadcast([TILE, ntiles, 2]),
            op=mybir.AluOpType.mult,
        )
        nc.vector.tensor_tensor(
            out=z[:],
            in0=z[:],
            in1=b2t[:].unsqueeze(1).to_broadcast([TILE, ntiles, 2]),
            op=mybir.AluOpType.add,
        )
        # log_softmax over the 2 columns
        m = outp.tile([TILE, ntiles], F32, tag="m")
        nc.vector.tensor_reduce(
            out=m[:], in_=z[:], axis=mybir.AxisListType.X, op=mybir.AluOpType.max
        )
        nc.vector.tensor_tensor(
            out=z[:],
            in0=z[:],
            in1=m[:].unsqueeze(-1).to_broadcast([TILE, ntiles, 2]),
            op=mybir.AluOpType.subtract,
        )
        ez = outp.tile([TILE, ntiles, 2], F32, tag="ez")
        nc.scalar.activation(
            out=ez[:], in_=z[:], func=mybir.ActivationFunctionType.Exp
        )
        ss = outp.tile([TILE, ntiles], F32, tag="ss")
        nc.vector.tensor_reduce(
            out=ss[:], in_=ez[:], axis=mybir.AxisListType.X, op=mybir.AluOpType.add
        )
        nc.scalar.activation(
            out=ss[:], in_=ss[:], func=mybir.ActivationFunctionType.Ln
        )
        yt = outp.tile([TILE, ntiles, 2], F32, tag="yt")
        nc.vector.tensor_tensor(
            out=yt[:],
            in0=z[:],
            in1=ss[:].unsqueeze(-1).to_broadcast([TILE, ntiles, 2]),
            op=mybir.AluOpType.subtract,
        )
        nc.sync.dma_start(
            out=y[:].rearrange("(t n) c -> n t c", n=TILE), in_=yt[:]
        )
    return nc


# ------------------------------------------------------------------- driver


def _run_gat(x, edge_index, W1, att_src1, att_dst1, b1, W2, att_src2, att_dst2, b2,
             n_cores=NC, timing=None):
    n_nodes, fdim = x.shape
    nh, ch = att_src1.shape

    src = np.concatenate([np.asarray(edge_index[0]), np.arange(n_nodes)]).astype(
        np.int64
    )
    dst = np.concatenate([np.asarray(edge_index[1]), np.arange(n_nodes)]).astype(
        np.int64
    )

    per, ntiles, padn, Dt, nblocks, slot_src, orders = _plan(
        src, dst, n_nodes, n_cores
    )
    groups = _groups(Dt)
    tile_of_blk = np.repeat(np.arange(ntiles), Dt)  # tile index per slot block

    W1 = np.asarray(W1, np.float32)
    att_src1 = np.asarray(att_src1, np.float32)
    att_dst1 = np.asarray(att_dst1, np.float32)
    W2 = np.asarray(W2, np.float32)
    att_src2 = np.asarray(att_src2, np.float32)
    att_dst2 = np.asarray(att_dst2, np.float32)

    # fused weights
    w_asrc1 = np.stack(
        [W1[:, h * ch : (h + 1) * ch] @ att_src1[h] for h in range(nh)], axis=1
    )  # [F, nh]
    w_adst1 = np.stack(
        [W1[:, h * ch : (h + 1) * ch] @ att_dst1[h] for h in range(nh)], axis=1
    )
    w1p = np.concatenate([W1, w_asrc1, w_adst1], axis=1).astype(NP16)  # [F, REC]
    w_asrc2 = W2 @ att_src2[0]
    w_adst2 = W2 @ att_dst2[0]
    w2p = np.concatenate(
        [W2, w_asrc2[:, None], w_adst2[:, None]], axis=1
    ).astype(NP16)  # [D1, 4]

    x = np.asarray(x, np.float32)

    # ---- launch A: per-node R1 ----
    in_maps0 = []
    for c in range(n_cores):
        ids = orders[c]
        real = ids >= 0
        xs = np.where(real[:, None], x[np.maximum(ids, 0)], 0.0)  # [padn, F]
        in_maps0.append(
            {"xt": np.ascontiguousarray(xs.T.astype(NP16)), "w1p": w1p}
        )
    nc0 = _build_a(padn, fdim)
    _split_waits(nc0)
    import time as _time

    t0 = _time.perf_counter()
    res0 = run_bass_kernel_spmd(nc0, in_maps0, list(range(n_cores)))
    t1 = _time.perf_counter()
    if timing is not None:
        timing["a_first_s"] = t1 - t0
        timing["nc0"] = nc0
        timing["in_maps0"] = in_maps0

    # R1 lookup table: [h1(32) | a_src(2) | a_dst(2)], pad row kills e
    r1tab = np.zeros((n_nodes + 1, REC), NP16)
    r1tab[n_nodes, D1 : D1 + NH] = BIG_NEG
    for c in range(n_cores):
        ids = orders[c]
        real = ids >= 0
        r1tab[ids[real]] = res0.results[c]["r1"][:, real].T

    # ---- launch B inputs: grouped channel-major slot planes ----
    in_maps1 = []
    b1r = np.broadcast_to(np.asarray(b1, NP16), (TILE, D1)).copy()
    for c in range(n_cores):
        chunks = []
        blk = 0
        for t0g, ng, cg in groups:
            g = r1tab[slot_src[c, blk : blk + cg]]  # [cg, TILE, REC]
            gt = g.transpose(1, 2, 0)  # [TILE, REC, cg]
            hp = gt[:, 0:D1, :]
            asr = gt[:, D1 : D1 + NH, :]
            ids = orders[c][t0g * TILE : (t0g + ng) * TILE].reshape(ng, TILE)
            adv = r1tab[np.maximum(ids, 0), D1 + NH : REC]  # [ng, TILE, NH]
            adv = adv * (ids >= 0)[:, :, None].astype(NP16)
            reps = Dt[t0g : t0g + ng].astype(np.int64)
            ade = np.repeat(adv, reps, axis=0)  # [cg, TILE, NH]
            ade = ade.transpose(1, 2, 0)  # [TILE, NH, cg]
            chunks.append(
                np.concatenate([hp, asr, ade], axis=1).reshape(TILE, 36 * cg)
            )
            blk += cg
        in_maps1.append(
            {
                "hsd": np.ascontiguousarray(np.concatenate(chunks, axis=1)),
                "b1r": b1r,
                "w2p": w2p,
            }
        )

    nc1 = _build_l1(Dt, groups, ntiles, padn)
    _split_waits(nc1)
    t2 = _time.perf_counter()
    res1 = run_bass_kernel_spmd(nc1, in_maps1, list(range(n_cores)))
    t3 = _time.perf_counter()
    if timing is not None:
        timing["l1_first_s"] = t3 - t2
        timing["nc1"] = nc1
        timing["in_maps1"] = in_maps1

    # R2 lookup table: [h2(2) | a_src2 | a_dst2]
    r2tab = np.zeros((n_nodes + 1, 4), NP16)
    r2tab[n_nodes, 2] = BIG_NEG
    for c in range(n_cores):
        ids = orders[c]
        real = ids >= 0
        r2tab[ids[real]] = res1.results[c]["r2"][real]

    # ---- launch C inputs: planar slots over the whole core ----
    in_maps2 = []
    b2r = np.broadcast_to(np.asarray(b2, np.float32), (TILE, 2)).copy()
    for c in range(n_cores):
        g = r2tab[slot_src[c]]  # [nblocks, TILE, 4]
        gt = g.transpose(1, 2, 0)  # [TILE, 4, nblocks]
        h2 = gt[:, 0:2, :].reshape(TILE, 2 * nblocks)
        as2 = gt[:, 2, :]
        ids = orders[c].reshape(ntiles, TILE)
        adv = r2tab[np.maximum(ids, 0), 3] * (ids >= 0).astype(NP16)  # [nt, TILE]
        ad2 = np.repeat(adv, Dt.astype(np.int64), axis=0).T  # [TILE, nblocks]
        xed = np.concatenate([h2, as2, ad2], axis=1)  # [TILE, 4*nblocks]
        in_maps2.append({"xed": np.ascontiguousarray(xed), "b2r": b2r})

    nc2 = _build_l2(Dt, ntiles, padn)
    _split_waits(nc2)
    t4 = _time.perf_counter()
    res2 = run_bass_kernel_spmd(nc2, in_maps2, list(range(n_cores)))
    t5 = _time.perf_counter()
    if timing is not None:
        timing["l2_first_s"] = t5 - t4
        timing["nc2"] = nc2
        timing["in_maps2"] = in_maps2

    out = np.zeros((n_nodes, 2), np.float32)
    for c in range(n_cores):
        yc = res2.results[c]["y"]
        ids = orders[c]
        real = ids >= 0
        out[ids[real]] = yc[real]
    return out


def kernel(x, edge_index, W1, att_src1, att_dst1, b1, W2, att_src2, att_dst2, b2):
    return _run_gat(
        np.asarray(x, np.float32),
        np.asarray(edge_index),
        W1,
        att_src1,
        att_dst1,
        b1,
        W2,
        att_src2,
        att_dst2,
        b2,
    )


# revision 10
# speedup vs baseline: 4.9590x; 2.1808x over previous
"""GAT (2-layer, PyG-style) on 8 Trainium2 NeuronCores.

Strategy (edge parallelism per the sharding hint), v3 — three launches:
  - Nodes are split into 8 contiguous ranges (12500/core); each core owns all
    in-edges of its nodes (~412K edges, uniform since the graph is random).
    Per-core nodes are degree-sorted into 128-lane tiles. Tiles are packed
    into groups with a UNIFORM slot depth Dg per group (pad slots reference a
    sentinel row whose a_src = -30000 so e underflows to exactly 0; ~3.5%
    padding thanks to the degree sort).
  - Launch A (node pass): R1 = [h1 | a_src1 | a_dst1] = x @ [W1|W1@As|W1@Ad]
    once per node on the PE (fp16, weights stationary, channel-major out).
  - Host gathers R1[src] per edge slot (72B/edge fp16; 3.5x less HBM than
    raw-x gather, no per-edge matmuls).
  - Launch B (layer-1 edge pass): per group, stream channel-major planes
    [h(32) | a_src(2) | a_dst(2)] fp16; e = exp(lrelu(a_src+a_dst)) (exp on
    ACT); V = e*h with one big packed-fp16 DVE mult (fast path ~0.19ns/elem);
    segment-sum via IN-PLACE fp16 halving-tree adds over the uniform slot
    axis (elementwise fp16 is ~6x faster than tensor_reduce on HW), final
    pair summed to f32. Channel-major finishing: normalize, +b1, ELU,
    R2 = eluT @ [W2|W2@As2|W2@Ad2] via PE transposes.
  - Host gathers R2[src] per edge slot (8B/edge); launch C does layer 2 the
    same way; with only 2 output channels the softmax/log_softmax needs no
    reduces at all (channel-pair tensor_tensor ops).
"""

import sys

sys.path.insert(0, "/opt/trn_rl_repo")

from contextlib import ExitStack

import numpy as np

import concourse.tile as tile
from concourse import bass, mybir
from concourse.bass_utils import run_bass_kernel_spmd
from concourse.masks import make_identity

F32 = mybir.dt.float32
F16 = mybir.dt.float16
NP16 = np.float16

NC = 8
TILE = 128
NH = 2
CH = 16
D1 = NH * CH  # 32
REC = D1 + 2 * NH  # 36
NEG_SLOPE = 0.2
BIG_NEG = -30000.0  # fp16-safe; 0.2*BIG_NEG underflows exp to exactly 0
L_BUDGET = 384  # max (padded) slot columns per group


_ws_seq = [0]


def _split_waits(nc, limit=1):
    """The walrus build in this container rejects instructions carrying more
    than one sem wait ("Too many sync wait commands"). Hoist excess waits
    onto NOP carriers inserted just before the instruction (same engine, same
    program order, so semantics are preserved)."""
    for f in nc.m.functions:
        for blk in f.blocks:
            il = list(blk.instructions)
            out = []
            changed = False
            for inst in il:
                si = inst.sync_info
                waits = list(si.on_wait) if (si and si.on_wait) else []
                if len(waits) > limit:
                    keep = waits[-limit:]
                    for w in waits[:-limit]:
                        _ws_seq[0] += 1
                        nop = mybir.InstNoOp(name=f"WS-{_ws_seq[0]}")
                        nop.engine = inst.engine
                        nop.sync_info = mybir.SyncInfo(on_wait=[w], on_update=[])
                        out.append(nop)
                    si.on_wait = keep
                    changed = True
                out.append(inst)
            if changed:
                blk.instructions = out


# ---------------------------------------------------------------- host prep


def _plan(src, dst, n_nodes, n_cores):
    """Node ranges, degree-sorted tiles, shared D_t schedule, slot src ids."""
    per = n_nodes // n_cores
    ntiles = (per + TILE - 1) // TILE
    padn = ntiles * TILE

    deg = np.bincount(dst, minlength=n_nodes)

    # edges sorted by dst, self-loop (src==dst) first within each segment
    order_e = np.lexsort((src != dst, dst))
    s_src = src[order_e]
    rowptr = np.zeros(n_nodes + 1, dtype=np.int64)
    np.cumsum(deg, out=rowptr[1:])

    orders = []  # per core: global node id per sorted slot lane (-1 = fake)
    Dt_all = np.zeros((n_cores, ntiles), dtype=np.int64)
    for c in range(n_cores):
        d = deg[c * per : (c + 1) * per]
        ids = np.concatenate(
            [c * per + np.arange(per), np.full(padn - per, -1, np.int64)]
        )
        dd = np.concatenate([d, np.zeros(padn - per, np.int64)])
        o = np.argsort(dd, kind="stable")
        orders.append(ids[o])
        Dt_all[c] = dd[o].reshape(ntiles, TILE).max(axis=1)
    Dt = Dt_all.max(axis=0)
    Dt = np.maximum(Dt, 1)  # avoid zero-size tiles
    nblocks = int(Dt.sum())

    # slot src ids per core: [nblocks, TILE] int64, pad = n_nodes
    slot_src = np.full((n_cores, nblocks, TILE), n_nodes, dtype=np.int64)
    for c in range(n_cores):
        ids = orders[c]
        b0 = 0
        for t in range(ntiles):
            D = int(Dt[t])
            nid = ids[t * TILE : (t + 1) * TILE]
            real = nid >= 0
            nid_c = np.where(real, nid, 0)
            degs = np.where(real, deg[nid_c], 0)
            jj = np.arange(D)[:, None]  # [D, TILE]
            valid = jj < degs[None, :]
            eidx = rowptr[nid_c][None, :] + np.minimum(jj, np.maximum(degs - 1, 0))
            vals = s_src[np.clip(eidx, 0, len(s_src) - 1)]
            slot_src[c, b0 : b0 + D] = np.where(valid, vals, n_nodes)
            b0 += D
    return per, ntiles, padn, Dt, nblocks, slot_src, orders


def _groups(Dt):
    """Pack degree-sorted tiles into groups with a UNIFORM padded slot depth.

    Returns [(t0, ng, Dg)]: tiles t0..t0+ng-1, each padded to Dg slots."""
    groups = []
    t0 = 0
    for t in range(len(Dt)):
        if (t - t0 + 1) * int(Dt[t]) > L_BUDGET and t > t0:
            groups.append((t0, t - t0, int(Dt[t - 1])))
            t0 = t
    groups.append((t0, len(Dt) - t0, int(Dt[-1])))
    return groups


def _padded_slots(slot_src_c, Dt, groups, n_nodes):
    """Per-group padded slot-src tables: [L_g, TILE] with sentinel pads."""
    out = []
    blk = 0
    for t0, ng, dg in groups:
        tab = np.full((ng, dg, TILE), n_nodes, dtype=np.int64)
        for i in range(ng):
            D = int(Dt[t0 + i])
            tab[i, 0:D] = slot_src_c[blk : blk + D]
            blk += D
        out.append(tab.reshape(ng * dg, TILE))
    return out


# ------------------------------------------------------- launch A (node pass)


def _build_a(padn, fdim, repeat=None):
    """R1 = [h1 | a_src1 | a_dst1] = w1p.T @ x, channel-major out [REC, padn]."""
    nc = bass.Bass("TRN2")
    xt = nc.declare_dram_parameter("xt", [fdim, padn], F16, isOutput=False)
    w1p = nc.declare_dram_parameter("w1p", [fdim, REC], F16, isOutput=False)
    r1 = nc.declare_dram_parameter("r1", [REC, padn], F16, isOutput=True)
    nt = padn // TILE

    with ExitStack() as ctx:
        tc = ctx.enter_context(tile.TileContext(nc))
        const = ctx.enter_context(tc.tile_pool(name="const", bufs=1))
        xp = ctx.enter_context(tc.tile_pool(name="xp", bufs=1))
        pp = ctx.enter_context(tc.tile_pool(name="pp", bufs=4, space="PSUM"))
        op = ctx.enter_context(tc.tile_pool(name="op", bufs=1))

        w1t = const.tile([fdim, REC], F16)
        nc.sync.dma_start(out=w1t[:], in_=w1p[:])

        if repeat:
            ctx.enter_context(tc.For_i(0, repeat, 1))
        xtile = xp.tile([fdim, padn], F16, tag="xt")
        nc.sync.dma_start(out=xtile[:], in_=xt[:])
        r1sb = op.tile([REC, padn], F16, tag="r1sb")
        p1 = None
        for t in range(nt):
            q = t % 4
            if q == 0:
                p1 = pp.tile([REC, 4 * TILE], F32, tag="p1")
            nc.tensor.matmul(
                out=p1[:, q * TILE : (q + 1) * TILE],
                lhsT=w1t[:],
                rhs=xtile[:, t * TILE : (t + 1) * TILE],
                start=True,
                stop=True,
            )
            if q == 3 or t == nt - 1:
                nc.vector.tensor_copy(
                    out=r1sb[:, (t - q) * TILE : (t + 1) * TILE],
                    in_=p1[:, 0 : (q + 1) * TILE],
                )
        nc.sync.dma_start(out=r1[:], in_=r1sb[:])
    return nc


# ------------------------------------------------------------- launch B (L1)


def _tree_sum(nc, slicer, dg):
    """In-place halving-tree sum over the last axis of a [..., dg] fp16 AP.
    ``slicer(a, b)`` returns the AP sliced to [..., a:b] on the last axis.
    Leaves the pairwise total in columns 0 and 1 (dg>=2) or 0 (dg==1)."""
    d = dg
    while d > 2:
        k = d // 2
        nc.vector.tensor_tensor(
            out=slicer(0, k),
            in0=slicer(0, k),
            in1=slicer(d - k, d),
            op=mybir.AluOpType.add,
        )
        d -= k
    return d  # 2 (or 1 if dg==1)


def _build_l1(Dt, groups, ntiles, padn, repeat=None):
    """Layer-1 edge pass from host-gathered channel-major slot planes."""
    cb = 36 * sum(ng * dg for _, ng, dg in groups)
    lgm = max(ng * dg for _, ng, dg in groups)
    nc = bass.Bass("TRN2")
    hsd = nc.declare_dram_parameter("hsd", [TILE, cb], F16, isOutput=False)
    b1r = nc.declare_dram_parameter("b1r", [TILE, D1], F16, isOutput=False)
    w2p = nc.declare_dram_parameter("w2p", [D1, 4], F16, isOutput=False)
    r2 = nc.declare_dram_parameter("r2", [padn, 4], F16, isOutput=True)

    with ExitStack() as ctx:
        tc = ctx.enter_context(tile.TileContext(nc))
        const = ctx.enter_context(tc.tile_pool(name="const", bufs=1))
        hspool = ctx.enter_context(tc.tile_pool(name="hs", bufs=2))
        wk = ctx.enter_context(tc.tile_pool(name="wk", bufs=2))
        vpool = ctx.enter_context(tc.tile_pool(name="vp", bufs=2))
        ppool = ctx.enter_context(tc.tile_pool(name="pp", bufs=2, space="PSUM"))
        rpool = ctx.enter_context(tc.tile_pool(name="rp", bufs=2, space="PSUM"))
        outp = ctx.enter_context(tc.tile_pool(name="op", bufs=1))

        b1t = const.tile([TILE, D1], F16)
        nc.sync.dma_start(out=b1t[:], in_=b1r[:])
        w2t = const.tile([D1, 4], F16)
        nc.sync.dma_start(out=w2t[:], in_=w2p[:])
        ident = const.tile([TILE, TILE], F16)
        make_identity(nc, ident[:])

        if repeat:
            ctx.enter_context(tc.For_i(0, repeat, 1))
        o1acc = outp.tile([TILE, NH, CH, ntiles], F32, tag="o1acc")
        sacc = outp.tile([TILE, NH, ntiles], F32, tag="sacc")
        off = 0
        for t0, ng, dg in groups:
            L = ng * dg
            hst = hspool.tile([TILE, 36, lgm], F16, tag="hst")
            nc.sync.dma_start(
                out=hst[:, :, 0:L],
                in_=hsd[:, off : off + 36 * L].rearrange("p (c j) -> p c j", c=36),
            )
            lg = wk.tile([TILE, NH, lgm], F16, tag="lg")
            nc.vector.tensor_tensor(
                out=lg[:, :, 0:L],
                in0=hst[:, D1 : D1 + NH, 0:L],
                in1=hst[:, D1 + NH : REC, 0:L],
                op=mybir.AluOpType.add,
            )
            ls = wk.tile([TILE, NH, lgm], F16, tag="ls")
            nc.vector.tensor_scalar_mul(
                out=ls[:, :, 0:L], in0=lg[:, :, 0:L], scalar1=NEG_SLOPE
            )
            nc.vector.tensor_tensor(
                out=lg[:, :, 0:L],
                in0=lg[:, :, 0:L],
                in1=ls[:, :, 0:L],
                op=mybir.AluOpType.max,
            )
            et = wk.tile([TILE, NH, lgm], F16, tag="et")
            nc.scalar.activation(
                out=et[:, :, 0:L],
                in_=lg[:, :, 0:L],
                func=mybir.ActivationFunctionType.Exp,
            )
            V = vpool.tile([TILE, NH, CH, lgm], F16, tag="V")
            nc.vector.tensor_tensor(
                out=V[:, :, :, 0:L],
                in0=hst[:, 0:D1, 0:L].rearrange("p (h c) j -> p h c j", h=NH),
                in1=et[:, :, 0:L].unsqueeze(2).to_broadcast([TILE, NH, CH, L]),
                op=mybir.AluOpType.mult,
            )
            # segment-sum: fp16 halving tree over j, then f32 pair-add out
            V5 = V[:, :, :, 0:L].rearrange("p h c (t j) -> p h c t j", j=dg)
            dv = _tree_sum(nc, lambda a, b: V5[:, :, :, :, a:b], dg)
            E5 = et[:, :, 0:L].rearrange("p h (t j) -> p h t j", j=dg)
            de = _tree_sum(nc, lambda a, b: E5[:, :, :, a:b], dg)
            if dv == 2:
                nc.vector.tensor_tensor(
                    out=o1acc[:, :, :, t0 : t0 + ng],
                    in0=V5[:, :, :, :, 0],
                    in1=V5[:, :, :, :, 1],
                    op=mybir.AluOpType.add,
                )
                nc.vector.tensor_tensor(
                    out=sacc[:, :, t0 : t0 + ng],
                    in0=E5[:, :, :, 0],
                    in1=E5[:, :, :, 1],
                    op=mybir.AluOpType.add,
                )
            else:
                nc.vector.tensor_copy(
                    out=o1acc[:, :, :, t0 : t0 + ng], in_=V5[:, :, :, :, 0]
                )
                nc.vector.tensor_copy(
                    out=sacc[:, :, t0 : t0 + ng], in_=E5[:, :, :, 0]
                )
            off += 36 * L

        # ---- batched channel-major finishing ----
        inv = outp.tile([TILE, NH, ntiles], F32, tag="inv")
        nc.vector.tensor_scalar_add(out=inv[:], in0=sacc[:], scalar1=1e-16)
        nc.vector.reciprocal(out=inv[:], in_=inv[:])
        invh = outp.tile([TILE, NH, ntiles], F16, tag="invh")
        nc.vector.tensor_copy(out=invh[:], in_=inv[:])
        o1f = outp.tile([TILE, NH, CH, ntiles], F16, tag="o1f")
        nc.vector.tensor_tensor(
            out=o1f[:],
            in0=o1acc[:],
            in1=invh[:].unsqueeze(2).to_broadcast([TILE, NH, CH, ntiles]),
            op=mybir.AluOpType.mult,
        )
        nc.vector.tensor_tensor(
            out=o1f[:],
            in0=o1f[:],
            in1=b1t[:]
            .rearrange("p (h c) -> p h c", h=NH)
            .unsqueeze(-1)
            .to_broadcast([TILE, NH, CH, ntiles]),
            op=mybir.AluOpType.add,
        )
        # elu = max(x,0) + exp(min(x,0)) - 1
        e1 = outp.tile([TILE, NH, CH, ntiles], F16, tag="e1")
        nc.vector.tensor_scalar_min(out=e1[:], in0=o1f[:], scalar1=0.0)
        nc.scalar.activation(
            out=e1[:], in_=e1[:], func=mybir.ActivationFunctionType.Exp
        )
        nc.vector.tensor_scalar_add(out=e1[:], in0=e1[:], scalar1=-1.0)
        nc.vector.tensor_scalar_max(out=o1f[:], in0=o1f[:], scalar1=0.0)
        nc.vector.tensor_tensor(
            out=o1f[:], in0=o1f[:], in1=e1[:], op=mybir.AluOpType.add
        )
        # R2 = [h2 | a_src2 | a_dst2] = elu_out @ w2p via PE transposes
        o1tsb = outp.tile([D1, padn], F16, tag="o1t")
        pt = None
        for t in range(ntiles):
            q = t % 4
            if q == 0:
                pt = ppool.tile([D1, 4 * TILE], F16, tag="pt")
            nc.tensor.transpose(
                out=pt[:, q * TILE : (q + 1) * TILE],
                in_=o1f[:, :, :, t].rearrange("p h c -> p (h c)"),
                identity=ident[:],
            )
            if q == 3 or t == ntiles - 1:
                nc.vector.tensor_copy(
                    out=o1tsb[:, (t - q) * TILE : (t + 1) * TILE],
                    in_=pt[:, 0 : (q + 1) * TILE],
                )
        r2all = outp.tile([TILE, ntiles, 4], F16, tag="r2all")
        r2p = None
        for t in range(ntiles):
            q = t % 32
            if q == 0:
                r2p = rpool.tile([TILE, 32 * 4], F32, tag="r2p")
            nc.tensor.matmul(
                out=r2p[:, q * 4 : (q + 1) * 4],
                lhsT=o1tsb[:, t * TILE : (t + 1) * TILE],
                rhs=w2t[:],
                start=True,
                stop=True,
            )
            if q == 31 or t == ntiles - 1:
                nc.vector.tensor_copy(
                    out=r2all[:, t - q : t + 1, :],
                    in_=r2p[:, 0 : (q + 1) * 4].rearrange("p (t c) -> p t c", c=4),
                )
        nc.sync.dma_start(
            out=r2[:].rearrange("(t n) c -> n t c", n=TILE), in_=r2all[:]
        )
    return nc


# ------------------------------------------------------------- launch C (L2)


def _build_l2(Dt, groups, ntiles, padn, repeat=None):
    """Layer 2 (1 head, 2 ch) from grouped planar [h2(2) | a_src2 | a_dst2]
    slots, plus bias and log_softmax (no reduces: channel-pair ops)."""
    cb = 4 * sum(ng * dg for _, ng, dg in groups)
    lgm = max(ng * dg for _, ng, dg in groups)
    nc = bass.Bass("TRN2")
    xed = nc.declare_dram_parameter("xed", [TILE, cb], F16, isOutput=False)
    b2r = nc.declare_dram_parameter("b2r", [TILE, 2], F32, isOutput=False)
    y = nc.declare_dram_parameter("y", [padn, 2], F32, isOutput=True)

    with ExitStack() as ctx:
        tc = ctx.enter_context(tile.TileContext(nc))
        const = ctx.enter_context(tc.tile_pool(name="const", bufs=1))
        xp = ctx.enter_context(tc.tile_pool(name="xp", bufs=2))
        wk = ctx.enter_context(tc.tile_pool(name="wk", bufs=2))
        outp = ctx.enter_context(tc.tile_pool(name="op", bufs=1))

        b2t = const.tile([TILE, 2], F32)
        nc.sync.dma_start(out=b2t[:], in_=b2r[:])

        if repeat:
            ctx.enter_context(tc.For_i(0, repeat, 1))
        acc2 = outp.tile([TILE, 2, ntiles], F32, tag="acc2")
        s2 = outp.tile([TILE, ntiles], F32, tag="s2")
        off = 0
        for t0, ng, dg in groups:
            L = ng * dg
            xe = xp.tile([TILE, 4, lgm], F16, tag="xe")
            nc.sync.dma_start(
                out=xe[:, :, 0:L],
                in_=xed[:, off : off + 4 * L].rearrange("p (c j) -> p c j", c=4),
            )
            lg = wk.tile([TILE, lgm], F16, tag="lg")
            nc.vector.tensor_tensor(
                out=lg[:, 0:L], in0=xe[:, 2, 0:L], in1=xe[:, 3, 0:L],
                op=mybir.AluOpType.add,
            )
            ls = wk.tile([TILE, lgm], F16, tag="ls")
            nc.vector.tensor_scalar_mul(
                out=ls[:, 0:L], in0=lg[:, 0:L], scalar1=NEG_SLOPE
            )
            nc.vector.tensor_tensor(
                out=lg[:, 0:L], in0=lg[:, 0:L], in1=ls[:, 0:L],
                op=mybir.AluOpType.max,
            )
            et = wk.tile([TILE, lgm], F16, tag="et")
            nc.scalar.activation(
                out=et[:, 0:L], in_=lg[:, 0:L],
                func=mybir.ActivationFunctionType.Exp,
            )
            V = wk.tile([TILE, 2, lgm], F16, tag="V")
            nc.vector.tensor_tensor(
                out=V[:, :, 0:L],
                in0=xe[:, 0:2, 0:L],
                in1=et[:, 0:L].unsqueeze(1).to_broadcast([TILE, 2, L]),
                op=mybir.AluOpType.mult,
            )
            V5 = V[:, :, 0:L].rearrange("p c (t j) -> p c t j", j=dg)
            dv = _tree_sum(nc, lambda a, b: V5[:, :, :, a:b], dg)
            E5 = et[:, 0:L].rearrange("p (t j) -> p t j", j=dg)
            de = _tree_sum(nc, lambda a, b: E5[:, :, a:b], dg)
            if dv == 2:
                nc.vector.tensor_tensor(
                    out=acc2[:, :, t0 : t0 + ng],
                    in0=V5[:, :, :, 0], in1=V5[:, :, :, 1],
                    op=mybir.AluOpType.add,
                )
                nc.vector.tensor_tensor(
                    out=s2[:, t0 : t0 + ng],
                    in0=E5[:, :, 0], in1=E5[:, :, 1],
                    op=mybir.AluOpType.add,
                )
            else:
                nc.vector.tensor_copy(
                    out=acc2[:, :, t0 : t0 + ng], in_=V5[:, :, :, 0]
                )
                nc.vector.tensor_copy(out=s2[:, t0 : t0 + ng], in_=E5[:, :, 0])
            off += 4 * L

        # ---- batched channel-major finishing (c=2: no reduces needed) ----
        nc.vector.tensor_scalar_add(out=s2[:], in0=s2[:], scalar1=1e-16)
        nc.vector.reciprocal(out=s2[:], in_=s2[:])
        z = outp.tile([TILE, 2, ntiles], F32, tag="z")
        nc.vector.tensor_tensor(
            out=z[:],
            in0=acc2[:],
            in1=s2[:].unsqueeze(1).to_broadcast([TILE, 2, ntiles]),
            op=mybir.AluOpType.mult,
        )
        nc.vector.tensor_tensor(
            out=z[:],
            in0=z[:],
            in1=b2t[:].unsqueeze(-1).to_broadcast([TILE, 2, ntiles]),
            op=mybir.AluOpType.add,
        )
        # log_softmax over the channel pair
        m = outp.tile([TILE, ntiles], F32, tag="m")
        nc.vector.tensor_tensor(
            out=m[:], in0=z[:, 0, :], in1=z[:, 1, :], op=mybir.AluOpType.max
        )
        nc.vector.tensor_tensor(
            out=z[:],
            in0=z[:],
            in1=m[:].unsqueeze(1).to_broadcast([TILE, 2, ntiles]),
            op=mybir.AluOpType.subtract,
        )
        ez = outp.tile([TILE, 2, ntiles], F32, tag="ez")
        nc.scalar.activation(
            out=ez[:], in_=z[:], func=mybir.ActivationFunctionType.Exp
        )
        ss = outp.tile([TILE, ntiles], F32, tag="ss")
        nc.vector.tensor_tensor(
            out=ss[:], in0=ez[:, 0, :], in1=ez[:, 1, :], op=mybir.AluOpType.add
        )
        nc.scalar.activation(
            out=ss[:], in_=ss[:], func=mybir.ActivationFunctionType.Ln
        )
        yt = outp.tile([TILE, 2, ntiles], F32, tag="yt")
        nc.vector.tensor_tensor(
            out=yt[:],
            in0=z[:],
            in1=ss[:].unsqueeze(1).to_broadcast([TILE, 2, ntiles]),
            op=mybir.AluOpType.subtract,
        )
        yt2 = outp.tile([TILE, ntiles, 2], F32, tag="yt2")
        nc.vector.tensor_copy(out=yt2[:], in_=yt[:].rearrange("p c t -> p t c"))
        nc.sync.dma_start(
            out=y[:].rearrange("(t n) c -> n t c", n=TILE), in_=yt2[:]
        )
    return nc


# ------------------------------------------------------------------- driver


def _run_gat(x, edge_index, W1, att_src1, att_dst1, b1, W2, att_src2, att_dst2, b2,
             n_cores=NC, timing=None):
    n_nodes, fdim = x.shape
    nh, ch = att_src1.shape

    src = np.concatenate([np.asarray(edge_index[0]), np.arange(n_nodes)]).astype(
        np.int64
    )
    dst = np.concatenate([np.asarray(edge_index[1]), np.arange(n_nodes)]).astype(
        np.int64
    )

    per, ntiles, padn, Dt, nblocks, slot_src, orders = _plan(
        src, dst, n_nodes, n_cores
    )
    groups = _groups(Dt)

    W1 = np.asarray(W1, np.float32)
    att_src1 = np.asarray(att_src1, np.float32)
    att_dst1 = np.asarray(att_dst1, np.float32)
    W2 = np.asarray(W2, np.float32)
    att_src2 = np.asarray(att_src2, np.float32)
    att_dst2 = np.asarray(att_dst2, np.float32)

    # fused weights
    w_asrc1 = np.stack(
        [W1[:, h * ch : (h + 1) * ch] @ att_src1[h] for h in range(nh)], axis=1
    )  # [F, nh]
    w_adst1 = np.stack(
        [W1[:, h * ch : (h + 1) * ch] @ att_dst1[h] for h in range(nh)], axis=1
    )
    w1p = np.concatenate([W1, w_asrc1, w_adst1], axis=1).astype(NP16)  # [F, REC]
    w_asrc2 = W2 @ att_src2[0]
    w_adst2 = W2 @ att_dst2[0]
    w2p = np.concatenate(
        [W2, w_asrc2[:, None], w_adst2[:, None]], axis=1
    ).astype(NP16)  # [D1, 4]

    x = np.asarray(x, np.float32)

    # ---- launch A: per-node R1 ----
    in_maps0 = []
    for c in range(n_cores):
        ids = orders[c]
        real = ids >= 0
        xs = np.where(real[:, None], x[np.maximum(ids, 0)], 0.0)  # [padn, F]
        in_maps0.append(
            {"xt": np.ascontiguousarray(xs.T.astype(NP16)), "w1p": w1p}
        )
    nc0 = _build_a(padn, fdim)
    _split_waits(nc0)
    import time as _time

    t0 = _time.perf_counter()
    res0 = run_bass_kernel_spmd(nc0, in_maps0, list(range(n_cores)))
    t1 = _time.perf_counter()
    if timing is not None:
        timing["a_first_s"] = t1 - t0
        timing["nc0"] = nc0
        timing["in_maps0"] = in_maps0

    # R1 lookup table: [h1(32) | a_src(2) | a_dst(2)], pad row kills e
    r1tab = np.zeros((n_nodes + 1, REC), NP16)
    r1tab[n_nodes, D1 : D1 + NH] = BIG_NEG
    for c in range(n_cores):
        ids = orders[c]
        real = ids >= 0
        r1tab[ids[real]] = res0.results[c]["r1"][:, real].T

    pslots = [
        _padded_slots(slot_src[c], Dt, groups, n_nodes) for c in range(n_cores)
    ]

    # ---- launch B inputs: grouped channel-major slot planes ----
    in_maps1 = []
    b1r = np.broadcast_to(np.asarray(b1, NP16), (TILE, D1)).copy()
    for c in range(n_cores):
        chunks = []
        for gi, (t0g, ng, dg) in enumerate(groups):
            g = r1tab[pslots[c][gi]]  # [L, TILE, REC]
            gt = g.transpose(1, 2, 0)  # [TILE, REC, L]
            hp = gt[:, 0:D1, :]
            asr = gt[:, D1 : D1 + NH, :]
            ids = orders[c][t0g * TILE : (t0g + ng) * TILE].reshape(ng, TILE)
            adv = r1tab[np.maximum(ids, 0), D1 + NH : REC]  # [ng, TILE, NH]
            adv = adv * (ids >= 0)[:, :, None].astype(NP16)
            ade = np.repeat(adv, dg, axis=0)  # [L, TILE, NH]
            ade = ade.transpose(1, 2, 0)  # [TILE, NH, L]
            chunks.append(
                np.concatenate([hp, asr, ade], axis=1).reshape(TILE, 36 * ng * dg)
            )
        in_maps1.append(
            {
                "hsd": np.ascontiguousarray(np.concatenate(chunks, axis=1)),
                "b1r": b1r,
                "w2p": w2p,
            }
        )

    nc1 = _build_l1(Dt, groups, ntiles, padn)
    _split_waits(nc1)
    t2 = _time.perf_counter()
    res1 = run_bass_kernel_spmd(nc1, in_maps1, list(range(n_cores)))
    t3 = _time.perf_counter()
    if timing is not None:
        timing["l1_first_s"] = t3 - t2
        timing["nc1"] = nc1
        timing["in_maps1"] = in_maps1

    # R2 lookup table: [h2(2) | a_src2 | a_dst2]
    r2tab = np.zeros((n_nodes + 1, 4), NP16)
    r2tab[n_nodes, 2] = BIG_NEG
    for c in range(n_cores):
        ids = orders[c]
        real = ids >= 0
        r2tab[ids[real]] = res1.results[c]["r2"][real]

    # ---- launch C inputs: grouped planar slots ----
    in_maps2 = []
    b2r = np.broadcast_to(np.asarray(b2, np.float32), (TILE, 2)).copy()
    for c in range(n_cores):
        chunks = []
        for gi, (t0g, ng, dg) in enumerate(groups):
            g = r2tab[pslots[c][gi]]  # [L, TILE, 4]
            gt = g.transpose(1, 2, 0)  # [TILE, 4, L]
            h2 = gt[:, 0:2, :]
            as2 = gt[:, 2:3, :]
            ids = orders[c][t0g * TILE : (t0g + ng) * TILE].reshape(ng, TILE)
            adv = r2tab[np.maximum(ids, 0), 3] * (ids >= 0).astype(NP16)
            ad2 = np.repeat(adv, dg, axis=0).T[:, None, :]  # [TILE, 1, L]
            chunks.append(
                np.concatenate([h2, as2, ad2], axis=1).reshape(TILE, 4 * ng * dg)
            )
        in_maps2.append(
            {
                "xed": np.ascontiguousarray(np.concatenate(chunks, axis=1)),
                "b2r": b2r,
            }
        )

    nc2 = _build_l2(Dt, groups, ntiles, padn)
    _split_waits(nc2)
    t4 = _time.perf_counter()
    res2 = run_bass_kernel_spmd(nc2, in_maps2, list(range(n_cores)))
    t5 = _time.perf_counter()
    if timing is not None:
        timing["l2_first_s"] = t5 - t4
        timing["nc2"] = nc2
        timing["in_maps2"] = in_maps2

    out = np.zeros((n_nodes, 2), np.float32)
    for c in range(n_cores):
        yc = res2.results[c]["y"]
        ids = orders[c]
        real = ids >= 0
        out[ids[real]] = yc[real]
    return out


def kernel(x, edge_index, W1, att_src1, att_dst1, b1, W2, att_src2, att_dst2, b2):
    return _run_gat(
        np.asarray(x, np.float32),
        np.asarray(edge_index),
        W1,
        att_src1,
        att_dst1,
        b1,
        W2,
        att_src2,
        att_dst2,
        b2,
    )


# revision 18
# speedup vs baseline: 5.5771x; 1.1246x over previous
"""GAT (2-layer, PyG-style) on 8 Trainium2 NeuronCores.

Strategy (edge parallelism per the sharding hint), v3 — three launches:
  - Nodes are split into 8 contiguous ranges (12500/core); each core owns all
    in-edges of its nodes (~412K edges, uniform since the graph is random).
    Per-core nodes are degree-sorted into 128-lane tiles. Tiles are packed
    into groups with a UNIFORM slot depth Dg per group (pad slots reference a
    sentinel row whose a_src = -30000 so e underflows to exactly 0; ~3.5%
    padding thanks to the degree sort).
  - Launch A (node pass): R1 = [h1 | a_src1 | a_dst1] = x @ [W1|W1@As|W1@Ad]
    once per node on the PE (fp16, weights stationary, channel-major out).
  - Host gathers R1[src] per edge slot (72B/edge fp16; 3.5x less HBM than
    raw-x gather, no per-edge matmuls).
  - Launch B (layer-1 edge pass): per group, stream channel-major planes
    [h(32) | a_src(2) | a_dst(2)] fp16; e = exp(lrelu(a_src+a_dst)) (exp on
    ACT); V = e*h with one big packed-fp16 DVE mult (fast path ~0.19ns/elem);
    segment-sum via IN-PLACE fp16 halving-tree adds over the uniform slot
    axis (elementwise fp16 is ~6x faster than tensor_reduce on HW), final
    pair summed to f32. Channel-major finishing: normalize, +b1, ELU,
    R2 = eluT @ [W2|W2@As2|W2@Ad2] via PE transposes.
  - Host gathers R2[src] per edge slot (8B/edge); launch C does layer 2 the
    same way; with only 2 output channels the softmax/log_softmax needs no
    reduces at all (channel-pair tensor_tensor ops).
"""

import sys

sys.path.insert(0, "/opt/trn_rl_repo")

from contextlib import ExitStack

import numpy as np

import concourse.tile as tile
from concourse import bass, mybir
from concourse.bass_utils import run_bass_kernel_spmd
from concourse.masks import make_identity

F32 = mybir.dt.float32
F16 = mybir.dt.float16
NP16 = np.float16

NC = 8
TILE = 128
NH = 2
CH = 16
D1 = NH * CH  # 32
REC = D1 + 2 * NH  # 36
NEG_SLOPE = 0.2
BIG_NEG = -30000.0  # fp16-safe; 0.2*BIG_NEG underflows exp to exactly 0
L_BUDGET = 384  # max (padded) slot columns per group


_ws_seq = [0]


def _split_waits(nc, limit=1):
    """The walrus build in this container rejects instructions carrying more
    than one sem wait ("Too many sync wait commands"). Hoist excess waits
    onto NOP carriers inserted just before the instruction (same engine, same
    program order, so semantics are preserved)."""
    for f in nc.m.functions:
        for blk in f.blocks:
            il = list(blk.instructions)
            out = []
            changed = False
            for inst in il:
                si = inst.sync_info
                waits = list(si.on_wait) if (si and si.on_wait) else []
                if len(waits) > limit:
                    keep = waits[-limit:]
                    for w in waits[:-limit]:
                        _ws_seq[0] += 1
                        nop = mybir.InstNoOp(name=f"WS-{_ws_seq[0]}")
                        nop.engine = inst.engine
                        nop.sync_info = mybir.SyncInfo(on_wait=[w], on_update=[])
                        out.append(nop)
                    si.on_wait = keep
                    changed = True
                out.append(inst)
            if changed:
                blk.instructions = out


# ---------------------------------------------------------------- host prep


def _plan(src, dst, n_nodes, n_cores):
    """Node ranges, degree-sorted tiles, shared D_t schedule, slot src ids."""
    per = n_nodes // n_cores
    ntiles = (per + TILE - 1) // TILE
    padn = ntiles * TILE

    deg = np.bincount(dst, minlength=n_nodes)

    # edges sorted by dst, self-loop (src==dst) first within each segment
    order_e = np.lexsort((src != dst, dst))
    s_src = src[order_e]
    rowptr = np.zeros(n_nodes + 1, dtype=np.int64)
    np.cumsum(deg, out=rowptr[1:])

    orders = []  # per core: global node id per sorted slot lane (-1 = fake)
    Dt_all = np.zeros((n_cores, ntiles), dtype=np.int64)
    for c in range(n_cores):
        d = deg[c * per : (c + 1) * per]
        ids = np.concatenate(
            [c * per + np.arange(per), np.full(padn - per, -1, np.int64)]
        )
        dd = np.concatenate([d, np.zeros(padn - per, np.int64)])
        o = np.argsort(dd, kind="stable")
        orders.append(ids[o])
        Dt_all[c] = dd[o].reshape(ntiles, TILE).max(axis=1)
    Dt = Dt_all.max(axis=0)
    Dt = np.maximum(Dt, 1)  # avoid zero-size tiles
    nblocks = int(Dt.sum())

    # slot src ids per core: [nblocks, TILE] int64, pad = n_nodes
    slot_src = np.full((n_cores, nblocks, TILE), n_nodes, dtype=np.int64)
    for c in range(n_cores):
        ids = orders[c]
        b0 = 0
        for t in range(ntiles):
            D = int(Dt[t])
            nid = ids[t * TILE : (t + 1) * TILE]
            real = nid >= 0
            nid_c = np.where(real, nid, 0)
            degs = np.where(real, deg[nid_c], 0)
            jj = np.arange(D)[:, None]  # [D, TILE]
            valid = jj < degs[None, :]
            eidx = rowptr[nid_c][None, :] + np.minimum(jj, np.maximum(degs - 1, 0))
            vals = s_src[np.clip(eidx, 0, len(s_src) - 1)]
            slot_src[c, b0 : b0 + D] = np.where(valid, vals, n_nodes)
            b0 += D
    return per, ntiles, padn, Dt, nblocks, slot_src, orders


def _groups(Dt):
    """Pack degree-sorted tiles into groups with a UNIFORM padded slot depth.

    Returns [(t0, ng, Dg)]: tiles t0..t0+ng-1, each padded to Dg slots."""
    groups = []
    t0 = 0
    for t in range(len(Dt)):
        if (t - t0 + 1) * int(Dt[t]) > L_BUDGET and t > t0:
            groups.append((t0, t - t0, int(Dt[t - 1])))
            t0 = t
    groups.append((t0, len(Dt) - t0, int(Dt[-1])))
    return groups


def _padded_slots(slot_src_c, Dt, groups, n_nodes):
    """Per-group padded slot-src tables: [L_g, TILE] with sentinel pads."""
    out = []
    blk = 0
    for t0, ng, dg in groups:
        tab = np.full((ng, dg, TILE), n_nodes, dtype=np.int64)
        for i in range(ng):
            D = int(Dt[t0 + i])
            tab[i, 0:D] = slot_src_c[blk : blk + D]
            blk += D
        out.append(tab.reshape(ng * dg, TILE))
    return out


# ------------------------------------------------------- launch A (node pass)


def _build_a(padn, fdim, repeat=None):
    """R1 = [h1 | a_src1 | a_dst1] = w1p.T @ x, channel-major out [REC, padn]."""
    nc = bass.Bass("TRN2")
    xt = nc.declare_dram_parameter("xt", [fdim, padn], F16, isOutput=False)
    w1p = nc.declare_dram_parameter("w1p", [fdim, REC], F16, isOutput=False)
    r1 = nc.declare_dram_parameter("r1", [REC, padn], F16, isOutput=True)
    nt = padn // TILE

    with ExitStack() as ctx:
        tc = ctx.enter_context(tile.TileContext(nc))
        const = ctx.enter_context(tc.tile_pool(name="const", bufs=1))
        xp = ctx.enter_context(tc.tile_pool(name="xp", bufs=1))
        pp = ctx.enter_context(tc.tile_pool(name="pp", bufs=4, space="PSUM"))
        op = ctx.enter_context(tc.tile_pool(name="op", bufs=1))

        w1t = const.tile([fdim, REC], F16)
        nc.sync.dma_start(out=w1t[:], in_=w1p[:])

        if repeat:
            ctx.enter_context(tc.For_i(0, repeat, 1))
        xtile = xp.tile([fdim, padn], F16, tag="xt")
        nc.sync.dma_start(out=xtile[:], in_=xt[:])
        r1sb = op.tile([REC, padn], F16, tag="r1sb")
        p1 = None
        for t in range(nt):
            q = t % 4
            if q == 0:
                p1 = pp.tile([REC, 4 * TILE], F32, tag="p1")
            nc.tensor.matmul(
                out=p1[:, q * TILE : (q + 1) * TILE],
                lhsT=w1t[:],
                rhs=xtile[:, t * TILE : (t + 1) * TILE],
                start=True,
                stop=True,
            )
            if q == 3 or t == nt - 1:
                nc.vector.tensor_copy(
                    out=r1sb[:, (t - q) * TILE : (t + 1) * TILE],
                    in_=p1[:, 0 : (q + 1) * TILE],
                )
        nc.sync.dma_start(out=r1[:], in_=r1sb[:])
    return nc


# ------------------------------------------------------------- launch B (L1)


def _pp_sum(nc, pool, tag, src, C, ng, dg, cap):
    """Halving-tree sum over the last axis of ``src`` [p, C, ng, dg] fp16,
    ping-ponging through rotating flat scratch tiles from ``pool`` (out-of-
    place: in-place strided adds serialize pathologically on HW; each level's
    destination is fully packed). Returns (ap, d) where ap[..., 0:d] (d<=2)
    holds the partial totals."""
    cur, d = src, dg
    while d > 2:
        k = d // 2
        odd = d - 2 * k
        flat = pool.tile([TILE, cap], F16, tag=tag)
        dst = flat[:, 0 : C * ng * (k + odd)].rearrange(
            "p (c t j) -> p c t j", c=C, t=ng
        )
        nc.vector.tensor_tensor(
            out=dst[:, :, :, 0:k],
            in0=cur[:, :, :, 0:k],
            in1=cur[:, :, :, k : 2 * k],
            op=mybir.AluOpType.add,
        )
        if odd:
            nc.vector.tensor_copy(
                out=dst[:, :, :, k : k + 1], in_=cur[:, :, :, 2 * k : d]
            )
        cur = dst
        d = k + odd
    return cur, d


def _build_l1(Dt, groups, ntiles, padn, repeat=None):
    """Layer-1 edge pass from host-gathered channel-major slot planes."""
    cb = 36 * sum(ng * dg for _, ng, dg in groups)
    lgm = max(ng * dg for _, ng, dg in groups)
    nc = bass.Bass("TRN2")
    hsd = nc.declare_dram_parameter("hsd", [TILE, cb], F16, isOutput=False)
    b1r = nc.declare_dram_parameter("b1r", [TILE, D1], F16, isOutput=False)
    w2p = nc.declare_dram_parameter("w2p", [D1, 4], F16, isOutput=False)
    r2 = nc.declare_dram_parameter("r2", [padn, 4], F16, isOutput=True)

    with ExitStack() as ctx:
        tc = ctx.enter_context(tile.TileContext(nc))
        const = ctx.enter_context(tc.tile_pool(name="const", bufs=1))
        hspool = ctx.enter_context(tc.tile_pool(name="hs", bufs=2))
        wk = ctx.enter_context(tc.tile_pool(name="wk", bufs=2))
        vpool = ctx.enter_context(tc.tile_pool(name="vp", bufs=2))
        ppw = ctx.enter_context(tc.tile_pool(name="ppw", bufs=2))
        ppe = ctx.enter_context(tc.tile_pool(name="ppe", bufs=2))
        ppool = ctx.enter_context(tc.tile_pool(name="pp", bufs=2, space="PSUM"))
        rpool = ctx.enter_context(tc.tile_pool(name="rp", bufs=2, space="PSUM"))
        outp = ctx.enter_context(tc.tile_pool(name="op", bufs=1))
        capw = D1 * max((ng * (dg // 2 + 1)) for _, ng, dg in groups)
        cape = NH * max((ng * (dg // 2 + 1)) for _, ng, dg in groups)

        b1t = const.tile([TILE, D1], F16)
        nc.sync.dma_start(out=b1t[:], in_=b1r[:])
        w2t = const.tile([D1, 4], F16)
        nc.sync.dma_start(out=w2t[:], in_=w2p[:])
        ident = const.tile([TILE, TILE], F16)
        make_identity(nc, ident[:])

        if repeat:
            ctx.enter_context(tc.For_i(0, repeat, 1))
        o1acc = outp.tile([TILE, NH, CH, ntiles], F32, tag="o1acc")
        sacc = outp.tile([TILE, NH, ntiles], F32, tag="sacc")
        off = 0
        for t0, ng, dg in groups:
            L = ng * dg
            hst = hspool.tile([TILE, 36, lgm], F16, tag="hst")
            nc.sync.dma_start(
                out=hst[:, :, 0:L],
                in_=hsd[:, off : off + 36 * L].rearrange("p (c j) -> p c j", c=36),
            )
            lg = wk.tile([TILE, NH, lgm], F16, tag="lg")
            nc.vector.tensor_tensor(
                out=lg[:, :, 0:L],
                in0=hst[:, D1 : D1 + NH, 0:L],
                in1=hst[:, D1 + NH : REC, 0:L],
                op=mybir.AluOpType.add,
            )
            ls = wk.tile([TILE, NH, lgm], F16, tag="ls")
            nc.vector.tensor_scalar_mul(
                out=ls[:, :, 0:L], in0=lg[:, :, 0:L], scalar1=NEG_SLOPE
            )
            nc.vector.tensor_tensor(
                out=lg[:, :, 0:L],
                in0=lg[:, :, 0:L],
                in1=ls[:, :, 0:L],
                op=mybir.AluOpType.max,
            )
            et = wk.tile([TILE, NH, lgm], F16, tag="et")
            nc.scalar.activation(
                out=et[:, :, 0:L],
                in_=lg[:, :, 0:L],
                func=mybir.ActivationFunctionType.Exp,
            )
            V = vpool.tile([TILE, NH, CH, lgm], F16, tag="V")
            nc.vector.tensor_tensor(
                out=V[:, :, :, 0:L],
                in0=hst[:, 0:D1, 0:L].rearrange("p (h c) j -> p h c j", h=NH),
                in1=et[:, :, 0:L].unsqueeze(2).to_broadcast([TILE, NH, CH, L]),
                op=mybir.AluOpType.mult,
            )
            # segment-sum: fp16 ping-pong halving tree over j, f32 pair-add
            # out. (h c) stays merged: 3 free dims keep the DVE fast path.
            V5 = V[:, :, :, 0:L].rearrange("p h c (t j) -> p (h c) t j", j=dg)
            Vf, dv = _pp_sum(nc, ppw, "vw", V5, D1, ng, dg, capw)
            E5 = et[:, :, 0:L].rearrange("p h (t j) -> p h t j", j=dg)
            Ef, de = _pp_sum(nc, ppe, "ew", E5, NH, ng, dg, cape)
            oacc = o1acc[:, :, :, t0 : t0 + ng].rearrange("p h c t -> p (h c) t")
            if dv == 2:
                nc.vector.tensor_tensor(
                    out=oacc,
                    in0=Vf[:, :, :, 0],
                    in1=Vf[:, :, :, 1],
                    op=mybir.AluOpType.add,
                )
            else:
                nc.vector.tensor_copy(out=oacc, in_=Vf[:, :, :, 0])
            if de == 2:
                nc.vector.tensor_tensor(
                    out=sacc[:, :, t0 : t0 + ng],
                    in0=Ef[:, :, :, 0],
                    in1=Ef[:, :, :, 1],
                    op=mybir.AluOpType.add,
                )
            else:
                nc.vector.tensor_copy(
                    out=sacc[:, :, t0 : t0 + ng], in_=Ef[:, :, :, 0]
                )
            off += 36 * L

        # ---- batched channel-major finishing ----
        inv = outp.tile([TILE, NH, ntiles], F32, tag="inv")
        nc.vector.tensor_scalar_add(out=inv[:], in0=sacc[:], scalar1=1e-16)
        nc.vector.reciprocal(out=inv[:], in_=inv[:])
        invh = outp.tile([TILE, NH, ntiles], F16, tag="invh")
        nc.vector.tensor_copy(out=invh[:], in_=inv[:])
        o1f = outp.tile([TILE, NH, CH, ntiles], F16, tag="o1f")
        nc.vector.tensor_tensor(
            out=o1f[:],
            in0=o1acc[:],
            in1=invh[:].unsqueeze(2).to_broadcast([TILE, NH, CH, ntiles]),
            op=mybir.AluOpType.mult,
        )
        nc.vector.tensor_tensor(
            out=o1f[:],
            in0=o1f[:],
            in1=b1t[:]
            .rearrange("p (h c) -> p h c", h=NH)
            .unsqueeze(-1)
            .to_broadcast([TILE, NH, CH, ntiles]),
            op=mybir.AluOpType.add,
        )
        # elu = max(x,0) + exp(min(x,0)) - 1
        e1 = outp.tile([TILE, NH, CH, ntiles], F16, tag="e1")
        nc.vector.tensor_scalar_min(out=e1[:], in0=o1f[:], scalar1=0.0)
        nc.scalar.activation(
            out=e1[:], in_=e1[:], func=mybir.ActivationFunctionType.Exp
        )
        nc.vector.tensor_scalar_add(out=e1[:], in0=e1[:], scalar1=-1.0)
        nc.vector.tensor_scalar_max(out=o1f[:], in0=o1f[:], scalar1=0.0)
        nc.vector.tensor_tensor(
            out=o1f[:], in0=o1f[:], in1=e1[:], op=mybir.AluOpType.add
        )
        # R2 = [h2 | a_src2 | a_dst2] = elu_out @ w2p via PE transposes
        o1tsb = outp.tile([D1, padn], F16, tag="o1t")
        pt = None
        for t in range(ntiles):
            q = t % 4
            if q == 0:
                pt = ppool.tile([D1, 4 * TILE], F16, tag="pt")
            nc.tensor.transpose(
                out=pt[:, q * TILE : (q + 1) * TILE],
                in_=o1f[:, :, :, t].rearrange("p h c -> p (h c)"),
                identity=ident[:],
            )
            if q == 3 or t == ntiles - 1:
                nc.vector.tensor_copy(
                    out=o1tsb[:, (t - q) * TILE : (t + 1) * TILE],
                    in_=pt[:, 0 : (q + 1) * TILE],
                )
        r2all = outp.tile([TILE, ntiles, 4], F16, tag="r2all")
        r2p = None
        for t in range(ntiles):
            q = t % 32
            if q == 0:
                r2p = rpool.tile([TILE, 32 * 4], F32, tag="r2p")
            nc.tensor.matmul(
                out=r2p[:, q * 4 : (q + 1) * 4],
                lhsT=o1tsb[:, t * TILE : (t + 1) * TILE],
                rhs=w2t[:],
                start=True,
                stop=True,
            )
            if q == 31 or t == ntiles - 1:
                nc.vector.tensor_copy(
                    out=r2all[:, t - q : t + 1, :],
                    in_=r2p[:, 0 : (q + 1) * 4].rearrange("p (t c) -> p t c", c=4),
                )
        nc.sync.dma_start(
            out=r2[:].rearrange("(t n) c -> n t c", n=TILE), in_=r2all[:]
        )
    return nc


# ------------------------------------------------------------- launch C (L2)


def _build_l2(Dt, groups, ntiles, padn, repeat=None):
    """Layer 2 (1 head, 2 ch) from grouped planar [h2(2) | a_src2 | a_dst2]
    slots, plus bias and log_softmax (no reduces: channel-pair ops)."""
    cb = 4 * sum(ng * dg for _, ng, dg in groups)
    lgm = max(ng * dg for _, ng, dg in groups)
    nc = bass.Bass("TRN2")
    xed = nc.declare_dram_parameter("xed", [TILE, cb], F16, isOutput=False)
    b2r = nc.declare_dram_parameter("b2r", [TILE, 2], F32, isOutput=False)
    y = nc.declare_dram_parameter("y", [padn, 2], F32, isOutput=True)

    with ExitStack() as ctx:
        tc = ctx.enter_context(tile.TileContext(nc))
        const = ctx.enter_context(tc.tile_pool(name="const", bufs=1))
        xp = ctx.enter_context(tc.tile_pool(name="xp", bufs=2))
        wk = ctx.enter_context(tc.tile_pool(name="wk", bufs=2))
        ppw = ctx.enter_context(tc.tile_pool(name="ppw", bufs=2))
        ppe = ctx.enter_context(tc.tile_pool(name="ppe", bufs=2))
        outp = ctx.enter_context(tc.tile_pool(name="op", bufs=1))
        capw = 2 * max((ng * (dg // 2 + 1)) for _, ng, dg in groups)
        cape = 1 * max((ng * (dg // 2 + 1)) for _, ng, dg in groups)

        b2t = const.tile([TILE, 2], F32)
        nc.sync.dma_start(out=b2t[:], in_=b2r[:])

        if repeat:
            ctx.enter_context(tc.For_i(0, repeat, 1))
        acc2 = outp.tile([TILE, 2, ntiles], F32, tag="acc2")
        s2 = outp.tile([TILE, ntiles], F32, tag="s2")
        off = 0
        for t0, ng, dg in groups:
            L = ng * dg
            xe = xp.tile([TILE, 4, lgm], F16, tag="xe")
            nc.sync.dma_start(
                out=xe[:, :, 0:L],
                in_=xed[:, off : off + 4 * L].rearrange("p (c j) -> p c j", c=4),
            )
            lg = wk.tile([TILE, lgm], F16, tag="lg")
            nc.vector.tensor_tensor(
                out=lg[:, 0:L], in0=xe[:, 2, 0:L], in1=xe[:, 3, 0:L],
                op=mybir.AluOpType.add,
            )
            ls = wk.tile([TILE, lgm], F16, tag="ls")
            nc.vector.tensor_scalar_mul(
                out=ls[:, 0:L], in0=lg[:, 0:L], scalar1=NEG_SLOPE
            )
            nc.vector.tensor_tensor(
                out=lg[:, 0:L], in0=lg[:, 0:L], in1=ls[:, 0:L],
                op=mybir.AluOpType.max,
            )
            et = wk.tile([TILE, lgm], F16, tag="et")
            nc.scalar.activation(
                out=et[:, 0:L], in_=lg[:, 0:L],
                func=mybir.ActivationFunctionType.Exp,
            )
            V = wk.tile([TILE, 2, lgm], F16, tag="V")
            nc.vector.tensor_tensor(
                out=V[:, :, 0:L],
                in0=xe[:, 0:2, 0:L],
                in1=et[:, 0:L].unsqueeze(1).to_broadcast([TILE, 2, L]),
                op=mybir.AluOpType.mult,
            )
            V5 = V[:, :, 0:L].rearrange("p c (t j) -> p c t j", j=dg)
            Vf, dv = _pp_sum(nc, ppw, "vw", V5, 2, ng, dg, capw)
            E5 = et[:, 0:L].rearrange("p (t j) -> p t j", j=dg).unsqueeze(1)
            Ef, de = _pp_sum(nc, ppe, "ew", E5, 1, ng, dg, cape)
            if dv == 2:
                nc.vector.tensor_tensor(
                    out=acc2[:, :, t0 : t0 + ng],
                    in0=Vf[:, :, :, 0], in1=Vf[:, :, :, 1],
                    op=mybir.AluOpType.add,
                )
            else:
                nc.vector.tensor_copy(
                    out=acc2[:, :, t0 : t0 + ng], in_=Vf[:, :, :, 0]
                )
            if de == 2:
                nc.vector.tensor_tensor(
                    out=s2[:, t0 : t0 + ng],
                    in0=Ef[:, 0, :, 0], in1=Ef[:, 0, :, 1],
                    op=mybir.AluOpType.add,
                )
            else:
                nc.vector.tensor_copy(out=s2[:, t0 : t0 + ng], in_=Ef[:, 0, :, 0])
            off += 4 * L

        # ---- batched channel-major finishing (c=2: no reduces needed) ----
        nc.vector.tensor_scalar_add(out=s2[:], in0=s2[:], scalar1=1e-16)
        nc.vector.reciprocal(out=s2[:], in_=s2[:])
        z = outp.tile([TILE, 2, ntiles], F32, tag="z")
        nc.vector.tensor_tensor(
            out=z[:],
            in0=acc2[:],
            in1=s2[:].unsqueeze(1).to_broadcast([TILE, 2, ntiles]),
            op=mybir.AluOpType.mult,
        )
        nc.vector.tensor_tensor(
            out=z[:],
            in0=z[:],
            in1=b2t[:].unsqueeze(-1).to_broadcast([TILE, 2, ntiles]),
            op=mybir.AluOpType.add,
        )
        # log_softmax over the channel pair
        m = outp.tile([TILE, ntiles], F32, tag="m")
        nc.vector.tensor_tensor(
            out=m[:], in0=z[:, 0, :], in1=z[:, 1, :], op=mybir.AluOpType.max
        )
        nc.vector.tensor_tensor(
            out=z[:],
            in0=z[:],
            in1=m[:].unsqueeze(1).to_broadcast([TILE, 2, ntiles]),
            op=mybir.AluOpType.subtract,
        )
        ez = outp.tile([TILE, 2, ntiles], F32, tag="ez")
        nc.scalar.activation(
            out=ez[:], in_=z[:], func=mybir.ActivationFunctionType.Exp
        )
        ss = outp.tile([TILE, ntiles], F32, tag="ss")
        nc.vector.tensor_tensor(
            out=ss[:], in0=ez[:, 0, :], in1=ez[:, 1, :], op=mybir.AluOpType.add
        )
        nc.scalar.activation(
            out=ss[:], in_=ss[:], func=mybir.ActivationFunctionType.Ln
        )
        yt = outp.tile([TILE, 2, ntiles], F32, tag="yt")
        nc.vector.tensor_tensor(
            out=yt[:],
            in0=z[:],
            in1=ss[:].unsqueeze(1).to_broadcast([TILE, 2, ntiles]),
            op=mybir.AluOpType.subtract,
        )
        yt2 = outp.tile([TILE, ntiles, 2], F32, tag="yt2")
        nc.vector.tensor_copy(out=yt2[:], in_=yt[:].rearrange("p c t -> p t c"))
        nc.sync.dma_start(
            out=y[:].rearrange("(t n) c -> n t c", n=TILE), in_=yt2[:]
        )
    return nc


# ------------------------------------------------------------------- driver


def _run_gat(x, edge_index, W1, att_src1, att_dst1, b1, W2, att_src2, att_dst2, b2,
             n_cores=NC, timing=None):
    n_nodes, fdim = x.shape
    nh, ch = att_src1.shape

    src = np.concatenate([np.asarray(edge_index[0]), np.arange(n_nodes)]).astype(
        np.int64
    )
    dst = np.concatenate([np.asarray(edge_index[1]), np.arange(n_nodes)]).astype(
        np.int64
    )

    per, ntiles, padn, Dt, nblocks, slot_src, orders = _plan(
        src, dst, n_nodes, n_cores
    )
    groups = _groups(Dt)

    W1 = np.asarray(W1, np.float32)
    att_src1 = np.asarray(att_src1, np.float32)
    att_dst1 = np.asarray(att_dst1, np.float32)
    W2 = np.asarray(W2, np.float32)
    att_src2 = np.asarray(att_src2, np.float32)
    att_dst2 = np.asarray(att_dst2, np.float32)

    # fused weights
    w_asrc1 = np.stack(
        [W1[:, h * ch : (h + 1) * ch] @ att_src1[h] for h in range(nh)], axis=1
    )  # [F, nh]
    w_adst1 = np.stack(
        [W1[:, h * ch : (h + 1) * ch] @ att_dst1[h] for h in range(nh)], axis=1
    )
    w1p = np.concatenate([W1, w_asrc1, w_adst1], axis=1).astype(NP16)  # [F, REC]
    w_asrc2 = W2 @ att_src2[0]
    w_adst2 = W2 @ att_dst2[0]
    w2p = np.concatenate(
        [W2, w_asrc2[:, None], w_adst2[:, None]], axis=1
    ).astype(NP16)  # [D1, 4]

    x = np.asarray(x, np.float32)

    # ---- launch A: per-node R1 ----
    in_maps0 = []
    for c in range(n_cores):
        ids = orders[c]
        real = ids >= 0
        xs = np.where(real[:, None], x[np.maximum(ids, 0)], 0.0)  # [padn, F]
        in_maps0.append(
            {"xt": np.ascontiguousarray(xs.T.astype(NP16)), "w1p": w1p}
        )
    nc0 = _build_a(padn, fdim)
    _split_waits(nc0)
    import time as _time

    t0 = _time.perf_counter()
    res0 = run_bass_kernel_spmd(nc0, in_maps0, list(range(n_cores)))
    t1 = _time.perf_counter()
    if timing is not None:
        timing["a_first_s"] = t1 - t0
        timing["nc0"] = nc0
        timing["in_maps0"] = in_maps0

    # R1 lookup table: [h1(32) | a_src(2) | a_dst(2)], pad row kills e
    r1tab = np.zeros((n_nodes + 1, REC), NP16)
    r1tab[n_nodes, D1 : D1 + NH] = BIG_NEG
    for c in range(n_cores):
        ids = orders[c]
        real = ids >= 0
        r1tab[ids[real]] = res0.results[c]["r1"][:, real].T

    pslots = [
        _padded_slots(slot_src[c], Dt, groups, n_nodes) for c in range(n_cores)
    ]

    # ---- launch B inputs: grouped channel-major slot planes ----
    in_maps1 = []
    b1r = np.broadcast_to(np.asarray(b1, NP16), (TILE, D1)).copy()
    for c in range(n_cores):
        chunks = []
        for gi, (t0g, ng, dg) in enumerate(groups):
            g = r1tab[pslots[c][gi]]  # [L, TILE, REC]
            gt = g.transpose(1, 2, 0)  # [TILE, REC, L]
            hp = gt[:, 0:D1, :]
            asr = gt[:, D1 : D1 + NH, :]
            ids = orders[c][t0g * TILE : (t0g + ng) * TILE].reshape(ng, TILE)
            adv = r1tab[np.maximum(ids, 0), D1 + NH : REC]  # [ng, TILE, NH]
            adv = adv * (ids >= 0)[:, :, None].astype(NP16)
            ade = np.repeat(adv, dg, axis=0)  # [L, TILE, NH]
            ade = ade.transpose(1, 2, 0)  # [TILE, NH, L]
            chunks.append(
                np.concatenate([hp, asr, ade], axis=1).reshape(TILE, 36 * ng * dg)
            )
        in_maps1.append(
            {
                "hsd": np.ascontiguousarray(np.concatenate(chunks, axis=1)),
                "b1r": b1r,
                "w2p": w2p,
            }
        )

    nc1 = _build_l1(Dt, groups, ntiles, padn)
    _split_waits(nc1)
    t2 = _time.perf_counter()
    res1 = run_bass_kernel_spmd(nc1, in_maps1, list(range(n_cores)))
    t3 = _time.perf_counter()
    if timing is not None:
        timing["l1_first_s"] = t3 - t2
        timing["nc1"] = nc1
        timing["in_maps1"] = in_maps1

    # R2 lookup table: [h2(2) | a_src2 | a_dst2]
    r2tab = np.zeros((n_nodes + 1, 4), NP16)
    r2tab[n_nodes, 2] = BIG_NEG
    for c in range(n_cores):
        ids = orders[c]
        real = ids >= 0
        r2tab[ids[real]] = res1.results[c]["r2"][real]

    # ---- launch C inputs: grouped planar slots ----
    in_maps2 = []
    b2r = np.broadcast_to(np.asarray(b2, np.float32), (TILE, 2)).copy()
    for c in range(n_cores):
        chunks = []
        for gi, (t0g, ng, dg) in enumerate(groups):
            g = r2tab[pslots[c][gi]]  # [L, TILE, 4]
            gt = g.transpose(1, 2, 0)  # [TILE, 4, L]
            h2 = gt[:, 0:2, :]
            as2 = gt[:, 2:3, :]
            ids = orders[c][t0g * TILE : (t0g + ng) * TILE].reshape(ng, TILE)
            adv = r2tab[np.maximum(ids, 0), 3] * (ids >= 0).astype(NP16)
            ad2 = np.repeat(adv, dg, axis=0).T[:, None, :]  # [TILE, 1, L]
            chunks.append(
                np.concatenate([h2, as2, ad2], axis=1).reshape(TILE, 4 * ng * dg)
            )
        in_maps2.append(
            {
                "xed": np.ascontiguousarray(np.concatenate(chunks, axis=1)),
                "b2r": b2r,
            }
        )

    nc2 = _build_l2(Dt, groups, ntiles, padn)
    _split_waits(nc2)
    t4 = _time.perf_counter()
    res2 = run_bass_kernel_spmd(nc2, in_maps2, list(range(n_cores)))
    t5 = _time.perf_counter()
    if timing is not None:
        timing["l2_first_s"] = t5 - t4
        timing["nc2"] = nc2
        timing["in_maps2"] = in_maps2

    out = np.zeros((n_nodes, 2), np.float32)
    for c in range(n_cores):
        yc = res2.results[c]["y"]
        ids = orders[c]
        real = ids >= 0
        out[ids[real]] = yc[real]
    return out


def kernel(x, edge_index, W1, att_src1, att_dst1, b1, W2, att_src2, att_dst2, b2):
    return _run_gat(
        np.asarray(x, np.float32),
        np.asarray(edge_index),
        W1,
        att_src1,
        att_dst1,
        b1,
        W2,
        att_src2,
        att_dst2,
        b2,
    )
